# revision 13
# baseline (speedup 1.0000x reference)
"""Trainium2 Bass kernel for nn_BlockAttention (block-local attention with RoPE + gate).

Sharding: sequence-parallel over 8 cores. Flattened [B*S=8192, E] rows split into
8 contiguous shards of 1024 rows (4 blocks of 256; blocks never cross cores or
batch boundaries since 4096/256=16 blocks per batch, 4 per core).

Per-core layout strategy (features-on-partitions, "transposed" activations):
  - host pre-transposes the x shard to xT [E, R] so no on-chip transposes needed
  - qT/kT/gateT [E, R] = W.T @ x via matmul(lhsT=W_chunk, rhs=xT_chunk)  (fp32r)
  - v [R, E] natural via matmul(lhsT=xT_chunk, rhs=Wv_chunk)
  - RoPE applied on transposed q/k with host-prepared cos/sin tables
    (replicated per head-pair, rotate-sign folded into sin table)
  - block-local attention per (block, head) with transposed scores S_T[k,q]:
    exp on ScalarE (no max subtraction needed: |scores/8| < ~15), row-sums via
    M=1 ones-matmul on PE, AV via matmul(lhsT=v_block, rhs=expS_T),
    softmax normalize via K=1 ones-outer-product replicate matmul + DVE mul
  - gate: sigmoid on ScalarE, fused multiply on DVE
  - out projection back through Wo in transposed layout; host un-transposes
"""
import sys

sys.path.insert(0, "/opt/trn_rl_repo")
import numpy as np

B, S, E = 2, 4096, 1024
H, D = 16, 64
BLK = 256
NCORES = 8
R = (B * S) // NCORES   # 1024 rows per core
NB = R // BLK           # 4 blocks per core
NCH = E // 128          # 8 feature chunks of 128
SCALE = 1.0 / np.sqrt(D)


def emit(tc, outs, ins):
    """Emit the per-core program. ins/outs are DRAM APs:
    ins  = [xT, wq, wk, wv, wg, wo, cos2, sin2]
    outs = [outT]
    """
    from contextlib import ExitStack
    import concourse.mybir as mybir

    F32 = mybir.dt.float32
    F32R = mybir.dt.float32r
    AF = mybir.ActivationFunctionType

    nc = tc.nc
    xT_d, wq_d, wk_d, wv_d, wg_d, wo_d, c2_d, s2_d = ins
    (outT_d,) = outs

    with ExitStack() as ctx:
        ep = ctx.enter_context
        consts = ep(tc.tile_pool(name="consts", bufs=1))
        big = ep(tc.tile_pool(name="big", bufs=1))
        wpool = ep(tc.tile_pool(name="wpool", bufs=2))
        wvpool = ep(tc.tile_pool(name="wvpool", bufs=1))
        ropet = ep(tc.tile_pool(name="ropet", bufs=2))
        espool = ep(tc.tile_pool(name="espool", bufs=2))
        smalls = ep(tc.tile_pool(name="smalls", bufs=2))
        rrsbp = ep(tc.tile_pool(name="rrsbp", bufs=2))
        ytp = ep(tc.tile_pool(name="ytp", bufs=2))
        opool = ep(tc.tile_pool(name="opool", bufs=1))
        # PSUM: 8 banks total, everything double-buffered.
        big_ps = ep(tc.tile_pool(name="big_ps", bufs=2, space="PSUM"))
        av_ps_p = ep(tc.tile_pool(name="av_ps_p", bufs=2, space="PSUM"))
        sum_ps_p = ep(tc.tile_pool(name="sum_ps_p", bufs=2, space="PSUM"))
        rr_ps_p = ep(tc.tile_pool(name="rr_ps_p", bufs=2, space="PSUM"))

        # ---- constants / inputs resident in SBUF
        xt = big.tile([128, NCH, R], F32R)
        for kc in range(NCH):
            nc.sync.dma_start(xt[:, kc, :],
                              xT_d[kc * 128:(kc + 1) * 128, :].bitcast(F32R))
        c2 = consts.tile([128, R], F32)
        nc.sync.dma_start(c2[:], c2_d[:])
        s2 = consts.tile([128, R], F32)
        nc.sync.dma_start(s2[:], s2_d[:])
        onesf = consts.tile([128, 1], F32)
        nc.vector.memset(onesf[:], 1.0)
        ones = consts.tile([128, 1], F32R)
        nc.scalar.activation(ones[:], onesf[:], AF.Copy)
        onesrowf = consts.tile([1, 64], F32)
        nc.vector.memset(onesrowf[:], 1.0)
        onesrow = consts.tile([1, 64], F32R)
        nc.scalar.activation(onesrow[:], onesrowf[:], AF.Copy)

        qT = big.tile([128, NCH, R], F32R)
        kT = big.tile([128, NCH, R], F32R)
        v = big.tile([128, NCH, R], F32R)
        # sg doubles as y: y1 multiplies the gate in-place (av*rr*sg),
        # and the out projection consumes it.
        sg = big.tile([128, NCH, R], F32R)

        # ---- one projection output chunk: 8-matmul psum group + drain
        def proj_chunk(w, dst, mc, nh, rope):
            ps = big_ps.tile([128, 512], F32, tag="big")
            for kc in range(NCH):
                nc.tensor.matmul(
                    ps[:],
                    w[:, kc, :],
                    xt[:, kc, nh * 512:(nh + 1) * 512],
                    start=(kc == 0),
                    stop=(kc == NCH - 1),
                )
            dstsl = dst[:, mc, nh * 512:(nh + 1) * 512]
            if rope:
                t = ropet.tile([128, 512], F32, tag="t")
                for h2 in (0, 64):
                    nc.scalar.activation(
                        t[h2:h2 + 32, :], ps[h2 + 32:h2 + 64, :], AF.Copy)
                    nc.scalar.activation(
                        t[h2 + 32:h2 + 64, :], ps[h2:h2 + 32, :], AF.Copy)
                nc.vector.tensor_mul(
                    dstsl, ps[:], c2[:, nh * 512:(nh + 1) * 512])
                nc.vector.tensor_mul(
                    t[:], t[:], s2[:, nh * 512:(nh + 1) * 512])
                nc.vector.tensor_add(dstsl, dstsl.bitcast(F32), t[:])
            else:
                nc.scalar.activation(dstsl, ps[:], AF.Sigmoid)

        def proj_load_w(w_d, mc):
            w = wpool.tile([128, NCH, 128], F32R, tag="w")
            src = w_d.rearrange("(kc p) m -> p kc m", p=128)
            nc.sync.dma_start(
                w[:], src[:, :, mc * 128:(mc + 1) * 128].bitcast(F32R))
            return w

        for mc in range(NCH):
            w = proj_load_w(wq_d, mc)
            for nh in range(2):
                proj_chunk(w, qT, mc, nh, rope=True)
        for mc in range(NCH):
            w = proj_load_w(wk_d, mc)
            for nh in range(2):
                proj_chunk(w, kT, mc, nh, rope=True)

        # ---- v projection (natural layout: rows on partitions)
        for nq in range(4):
            wvb = wvpool.tile([128, NCH, 256], F32R, tag="wv")
            for kc in range(NCH):
                nc.sync.dma_start(
                    wvb[:, kc, :],
                    wv_d[kc * 128:(kc + 1) * 128,
                         nq * 256:(nq + 1) * 256].bitcast(F32R))
            for rc in range(NCH):
                ps = big_ps.tile([128, 512], F32, tag="big")
                for kc in range(NCH):
                    nc.tensor.matmul(
                        ps[:, 0:256],
                        xt[:, kc, rc * 128:(rc + 1) * 128],
                        wvb[:, kc, :],
                        start=(kc == 0),
                        stop=(kc == NCH - 1),
                    )
                nc.vector.tensor_copy(
                    v[:, rc, nq * 256:(nq + 1) * 256], ps[:, 0:256])

        # ---- gate projection interleaved with block-local attention:
        # chunk c's gate lands just before attention needs sg[:, c, :],
        # and the gate matmuls keep PE dense while attention's ACT/DVE
        # chain (exp, recip, rr) runs.
        def attention(b, c):
            pair = []
            for hi in range(2):
                pb = 64 * hi
                sps = big_ps.tile([128, 512], F32, tag="big")
                for kph in range(2):
                    nc.tensor.matmul(
                        sps[:, kph * 256:(kph + 1) * 256],
                        kT[pb:pb + 64, c,
                           b * 256 + kph * 128:b * 256 + (kph + 1) * 128],
                        qT[pb:pb + 64, c, b * 256:(b + 1) * 256],
                        start=True, stop=True,
                    )
                pair.append(sps)
            est = []
            for hi in range(2):
                es = espool.tile([128, 512], F32R, tag="es")
                nc.scalar.activation(es[:], pair[hi][:], AF.Exp,
                                     scale=float(SCALE))
                est.append(es)
            recipf = smalls.tile([1, 512], F32, tag="recipf")
            avt = []
            for hi in range(2):
                es = est[hi]
                sums = sum_ps_p.tile([1, 256], F32, tag="sums")
                for kph in range(2):
                    nc.tensor.matmul(
                        sums[:],
                        ones[:],
                        es[:, kph * 256:(kph + 1) * 256],
                        start=(kph == 0), stop=(kph == 1),
                    )
                av = av_ps_p.tile([64, 256], F32, tag="av")
                for kph in range(2):
                    nc.tensor.matmul(
                        av[:],
                        v[:, 2 * b + kph, (2 * c + hi) * 64:(2 * c + hi + 1) * 64],
                        es[:, kph * 256:(kph + 1) * 256],
                        start=(kph == 0), stop=(kph == 1),
                    )
                avt.append(av)
                nc.vector.reciprocal_approx_fast(
                    recipf[0:1, hi * 256:(hi + 1) * 256], sums[:])
            recip = smalls.tile([1, 512], F32R, tag="recip")
            nc.scalar.activation(recip[:], recipf[:], AF.Copy)
            rr = rr_ps_p.tile([64, 512], F32, tag="rr")
            for hi in range(2):
                nc.tensor.matmul(
                    rr[:, hi * 256:(hi + 1) * 256],
                    onesrow[:],
                    recip[0:1, hi * 256:(hi + 1) * 256],
                    start=True, stop=True,
                )
            rrsb = rrsbp.tile([128, 256], F32, tag="rrsb")
            nc.scalar.activation(rrsb[0:64, :], rr[0:64, 0:256], AF.Copy)
            nc.scalar.activation(rrsb[64:128, :], rr[0:64, 256:512], AF.Copy)
            # y = av * rr * sigmoid(gate), written in-place over sg
            yt = ytp.tile([128, 256], F32, tag="yt")
            for hi in range(2):
                ysl_p = slice(64 * hi, 64 * hi + 64)
                nc.vector.tensor_mul(yt[ysl_p, :], avt[hi][:],
                                     rrsb[ysl_p, :])
                ysl = sg[ysl_p, c, b * 256:(b + 1) * 256]
                nc.vector.tensor_mul(ysl, ysl.bitcast(F32), yt[ysl_p, :])

        for c in range(NCH):
            w = proj_load_w(wg_d, c)
            for nh in range(2):
                proj_chunk(w, sg, c, nh, rope=False)
            for b in range(NB):
                attention(b, c)

        # ---- output projection (transposed): outT[of, r] = Wo.T @ y
        for oc in range(NCH):
            w = proj_load_w(wo_d, oc)
            for nh in range(2):
                ps = big_ps.tile([128, 512], F32, tag="big")
                for yc in range(NCH):
                    nc.tensor.matmul(
                        ps[:],
                        w[:, yc, :],
                        sg[:, yc, nh * 512:(nh + 1) * 512],
                        start=(yc == 0),
                        stop=(yc == NCH - 1),
                    )
                osb = opool.tile([128, 512], F32, tag="o")
                nc.scalar.activation(osb[:], ps[:], AF.Copy)
                nc.sync.dma_start(
                    outT_d[oc * 128:(oc + 1) * 128,
                           nh * 512:(nh + 1) * 512], osb[:])


def _build_nc():
    import concourse.bacc as bacc
    import concourse.mybir as mybir
    import concourse.tile as tile

    F32 = mybir.dt.float32
    nc = bacc.Bacc("TRN2", target_bir_lowering=False, debug=False)
    names_in = ["xT", "wq", "wk", "wv", "wg", "wo", "cos2", "sin2"]
    shapes_in = [[E, R], [E, E], [E, E], [E, E], [E, E], [E, E],
                 [128, R], [128, R]]
    ins = [
        nc.dram_tensor(n, s, F32, kind="ExternalInput").ap()
        for n, s in zip(names_in, shapes_in)
    ]
    outT = nc.dram_tensor("outT", [E, R], F32, kind="ExternalOutput").ap()
    with tile.TileContext(nc) as tc:
        emit(tc, [outT], ins)
    nc.compile()
    return nc


_NC_CACHE = {}


def host_prep(x, Wq, Wk, Wv, Wg, Wo, cos, sin):
    """Build the 8 per-core input maps."""
    x_flat = np.ascontiguousarray(x.reshape(B * S, E), dtype=np.float32)
    Wq = np.ascontiguousarray(Wq, dtype=np.float32)
    Wk = np.ascontiguousarray(Wk, dtype=np.float32)
    Wv = np.ascontiguousarray(Wv, dtype=np.float32)
    Wg = np.ascontiguousarray(Wg, dtype=np.float32)
    Wo = np.ascontiguousarray(Wo, dtype=np.float32)
    cos = np.asarray(cos, dtype=np.float32)
    sin = np.asarray(sin, dtype=np.float32)
    sign = np.where(np.arange(D) < D // 2, -1.0, 1.0).astype(np.float32)

    in_maps = []
    for cix in range(NCORES):
        rows = slice(cix * R, (cix + 1) * R)
        xT = np.ascontiguousarray(x_flat[rows].T)
        seq = (cix * R + np.arange(R)) % S
        cS = cos[seq]            # [R, D]
        sS = sin[seq] * sign     # [R, D] signed
        c2 = np.ascontiguousarray(np.tile(cS.T, (2, 1)))   # [128, R]
        s2 = np.ascontiguousarray(np.tile(sS.T, (2, 1)))   # [128, R]
        in_maps.append({
            "xT": xT, "wq": Wq, "wk": Wk, "wv": Wv, "wg": Wg, "wo": Wo,
            "cos2": c2, "sin2": s2,
        })
    return in_maps


def kernel_traced(x, Wq, Wk, Wv, Wg, Wo, cos, sin, block_size, trace=False,
                  **run_kwargs):
    assert int(block_size) == BLK
    from concourse import bass_utils

    if "nc" not in _NC_CACHE:
        _NC_CACHE["nc"] = _build_nc()
    nc = _NC_CACHE["nc"]

    in_maps = host_prep(x, Wq, Wk, Wv, Wg, Wo, cos, sin)
    res = bass_utils.run_bass_kernel_spmd(
        nc, in_maps, core_ids=list(range(NCORES)), trace=trace, **run_kwargs)
    out_flat = np.empty((B * S, E), dtype=np.float32)
    for cix in range(NCORES):
        out_flat[cix * R:(cix + 1) * R] = res.results[cix]["outT"].T
    return out_flat.reshape(B, S, E), res


def kernel(x, Wq, Wk, Wv, Wg, Wo, cos, sin, block_size):
    return kernel_traced(x, Wq, Wk, Wv, Wg, Wo, cos, sin, block_size)[0]


# revision 18
# speedup vs baseline: 1.0034x; 1.0034x over previous
"""Trainium2 Bass kernel for nn_BlockAttention (block-local attention with RoPE + gate).

Sharding: sequence-parallel over 8 cores. Flattened [B*S=8192, E] rows split into
8 contiguous shards of 1024 rows (4 blocks of 256; blocks never cross cores or
batch boundaries since 4096/256=16 blocks per batch, 4 per core).

Per-core layout strategy (features-on-partitions, "transposed" activations):
  - host pre-transposes the x shard to xT [E, R] so no on-chip transposes needed
  - qT/kT/gateT [E, R] = W.T @ x via matmul(lhsT=W_chunk, rhs=xT_chunk)  (fp32r)
  - v [R, E] natural via matmul(lhsT=xT_chunk, rhs=Wv_chunk)
  - RoPE applied on transposed q/k with host-prepared cos/sin tables
    (replicated per head-pair, rotate-sign folded into sin table)
  - block-local attention per (block, head) with transposed scores S_T[k,q]:
    exp on ScalarE (no max subtraction needed: |scores/8| < ~15), row-sums via
    M=1 ones-matmul on PE, AV via matmul(lhsT=v_block, rhs=expS_T),
    softmax normalize via K=1 ones-outer-product replicate matmul + DVE mul
  - gate: sigmoid on ScalarE, fused multiply on DVE
  - out projection back through Wo in transposed layout; host un-transposes
"""
import sys

sys.path.insert(0, "/opt/trn_rl_repo")
import numpy as np

B, S, E = 2, 4096, 1024
H, D = 16, 64
BLK = 256
NCORES = 8
R = (B * S) // NCORES   # 1024 rows per core
NB = R // BLK           # 4 blocks per core
NCH = E // 128          # 8 feature chunks of 128
SCALE = 1.0 / np.sqrt(D)


def emit(tc, outs, ins):
    """Emit the per-core program. ins/outs are DRAM APs:
    ins  = [xT, wq, wk, wv, wg, wo, cos2, sin2]
    outs = [outT]
    """
    from contextlib import ExitStack
    import concourse.mybir as mybir

    F32 = mybir.dt.float32
    F32R = mybir.dt.float32r
    AF = mybir.ActivationFunctionType

    nc = tc.nc
    xT_d, wq_d, wk_d, wv_d, wg_d, wo_d, c2_d, s2_d = ins
    (outT_d,) = outs

    with ExitStack() as ctx:
        ep = ctx.enter_context
        consts = ep(tc.tile_pool(name="consts", bufs=1))
        big = ep(tc.tile_pool(name="big", bufs=1))
        wpool = ep(tc.tile_pool(name="wpool", bufs=2))
        wvpool = ep(tc.tile_pool(name="wvpool", bufs=1))
        ropet = ep(tc.tile_pool(name="ropet", bufs=2))
        espool = ep(tc.tile_pool(name="espool", bufs=3))
        smalls = ep(tc.tile_pool(name="smalls", bufs=2))
        rrsbp = ep(tc.tile_pool(name="rrsbp", bufs=1))
        ytp = ep(tc.tile_pool(name="ytp", bufs=1))
        opool = ep(tc.tile_pool(name="opool", bufs=1))
        # PSUM: 8 banks total, everything double-buffered.
        big_ps = ep(tc.tile_pool(name="big_ps", bufs=2, space="PSUM"))
        s_ps_p = ep(tc.tile_pool(name="s_ps_p", bufs=2, space="PSUM"))
        av_ps_p = ep(tc.tile_pool(name="av_ps_p", bufs=2, space="PSUM"))
        rr_ps_p = ep(tc.tile_pool(name="rr_ps_p", bufs=2, space="PSUM"))

        # ---- constants / inputs resident in SBUF
        xt = big.tile([128, NCH, R], F32R)
        for kc in range(NCH):
            nc.sync.dma_start(xt[:, kc, :],
                              xT_d[kc * 128:(kc + 1) * 128, :].bitcast(F32R))
        c2 = consts.tile([128, R], F32)
        nc.sync.dma_start(c2[:], c2_d[:])
        s2 = consts.tile([128, R], F32)
        nc.sync.dma_start(s2[:], s2_d[:])
        onesf = consts.tile([128, 1], F32)
        nc.vector.memset(onesf[:], 1.0)
        ones = consts.tile([128, 1], F32R)
        nc.scalar.activation(ones[:], onesf[:], AF.Copy)
        onesrowf = consts.tile([1, 64], F32)
        nc.vector.memset(onesrowf[:], 1.0)
        onesrow = consts.tile([1, 64], F32R)
        nc.scalar.activation(onesrow[:], onesrowf[:], AF.Copy)

        qT = big.tile([128, NCH, R], F32R)
        kT = big.tile([128, NCH, R], F32R)
        # v holds 16 heads x (64 dims + a ones column) per row-chunk: the
        # ones column makes each AV matmul also emit the softmax row-sums
        # (output row 64) for free.
        v = big.tile([128, NCH, H * 65], F32R)
        ones16f = consts.tile([128, 16], F32)
        nc.vector.memset(ones16f[:], 1.0)
        # sg doubles as y: y1 multiplies the gate in-place (av*rr*sg),
        # and the out projection consumes it.
        sg = big.tile([128, NCH, R], F32R)

        # ---- one projection output chunk: 8-matmul psum group + drain
        def proj_chunk(w, dst, mc, nh, rope):
            ps = big_ps.tile([128, 512], F32, tag="big")
            for kc in range(NCH):
                nc.tensor.matmul(
                    ps[:],
                    w[:, kc, :],
                    xt[:, kc, nh * 512:(nh + 1) * 512],
                    start=(kc == 0),
                    stop=(kc == NCH - 1),
                )
            dstsl = dst[:, mc, nh * 512:(nh + 1) * 512]
            if rope:
                t = ropet.tile([128, 512], F32, tag="t")
                for h2 in (0, 64):
                    nc.scalar.activation(
                        t[h2:h2 + 32, :], ps[h2 + 32:h2 + 64, :], AF.Copy)
                    nc.scalar.activation(
                        t[h2 + 32:h2 + 64, :], ps[h2:h2 + 32, :], AF.Copy)
                nc.vector.tensor_mul(
                    dstsl, ps[:], c2[:, nh * 512:(nh + 1) * 512])
                nc.vector.tensor_mul(
                    t[:], t[:], s2[:, nh * 512:(nh + 1) * 512])
                nc.vector.tensor_add(dstsl, dstsl.bitcast(F32), t[:])
            else:
                nc.scalar.activation(dstsl, ps[:], AF.Sigmoid)

        def proj_load_w(w_d, mc):
            w = wpool.tile([128, NCH, 128], F32R, tag="w")
            src = w_d.rearrange("(kc p) m -> p kc m", p=128)
            nc.sync.dma_start(
                w[:], src[:, :, mc * 128:(mc + 1) * 128].bitcast(F32R))
            return w

        for mc in range(NCH):
            w = proj_load_w(wq_d, mc)
            for nh in range(2):
                proj_chunk(w, qT, mc, nh, rope=True)
        for mc in range(NCH):
            w = proj_load_w(wk_d, mc)
            for nh in range(2):
                proj_chunk(w, kT, mc, nh, rope=True)

        # ---- v projection (natural layout: rows on partitions)
        for nq in range(4):
            wvb = wvpool.tile([128, NCH, 256], F32R, tag="wv")
            for kc in range(NCH):
                nc.sync.dma_start(
                    wvb[:, kc, :],
                    wv_d[kc * 128:(kc + 1) * 128,
                         nq * 256:(nq + 1) * 256].bitcast(F32R))
            for rc in range(NCH):
                ps = big_ps.tile([128, 512], F32, tag="big")
                for kc in range(NCH):
                    nc.tensor.matmul(
                        ps[:, 0:256],
                        xt[:, kc, rc * 128:(rc + 1) * 128],
                        wvb[:, kc, :],
                        start=(kc == 0),
                        stop=(kc == NCH - 1),
                    )
                vh = v[:, rc, :].rearrange("p (h t) -> p h t", t=65)
                nc.vector.tensor_copy(
                    vh[:, 4 * nq:4 * nq + 4, 0:64],
                    ps[:, 0:256].rearrange("p (h d) -> p h d", d=64))
        for rc in range(NCH):
            vh = v[:, rc, :].rearrange("p (h t) -> p h t", t=65)
            nc.scalar.activation(vh[:, :, 64], ones16f[:], AF.Copy)

        # ---- gate projection interleaved with block-local attention:
        # chunk c's gate lands just before attention needs sg[:, c, :],
        # and the gate matmuls keep PE dense while attention's ACT/DVE
        # chain (exp, recip, rr) runs.
        def attn_front(b, c):
            est = []
            for hi in range(2):
                pb = 64 * hi
                sps = s_ps_p.tile([128, 512], F32, tag="s")
                for kph in range(2):
                    nc.tensor.matmul(
                        sps[:, kph * 256:(kph + 1) * 256],
                        kT[pb:pb + 64, c,
                           b * 256 + kph * 128:b * 256 + (kph + 1) * 128],
                        qT[pb:pb + 64, c, b * 256:(b + 1) * 256],
                        start=True, stop=True,
                    )
                es = espool.tile([128, 512], F32R, tag="es")
                nc.scalar.activation(es[:], sps[:], AF.Exp,
                                     scale=float(SCALE))
                est.append(es)
            return (b, c, est)

        def attn_back(st):
            b, c, est = st
            recipf = smalls.tile([1, 512], F32, tag="recipf", bufs=1)
            sumst = smalls.tile([1, 512], F32, tag="sumst", bufs=1)
            avt = []
            for hi in range(2):
                es = est[hi]
                h = 2 * c + hi
                # AV + row-sums in one accumulation group (ones-padded V):
                # rows 0..63 = V.T @ expS, row 64 = column sums of expS.
                av = av_ps_p.tile([65, 256], F32, tag="av")
                for kph in range(2):
                    nc.tensor.matmul(
                        av[:],
                        v[:, 2 * b + kph, h * 65:(h + 1) * 65],
                        es[:, kph * 256:(kph + 1) * 256],
                        start=(kph == 0), stop=(kph == 1),
                    )
                avt.append(av)
                # reciprocal_approx_fast misreads PSUM at base partition 64
                # on HW, so stage the sums row through SBUF.
                nc.vector.tensor_copy(
                    sumst[0:1, hi * 256:(hi + 1) * 256], av[64:65, :])
            nc.vector.reciprocal_approx_fast(recipf[:], sumst[:])
            recip = smalls.tile([1, 512], F32R, tag="recip")
            nc.scalar.activation(recip[:], recipf[:], AF.Copy)
            rr = rr_ps_p.tile([64, 512], F32, tag="rr")
            for hi in range(2):
                nc.tensor.matmul(
                    rr[:, hi * 256:(hi + 1) * 256],
                    onesrow[:],
                    recip[0:1, hi * 256:(hi + 1) * 256],
                    start=True, stop=True,
                )
            rrsb = rrsbp.tile([128, 256], F32, tag="rrsb")
            nc.scalar.activation(rrsb[0:64, :], rr[0:64, 0:256], AF.Copy)
            nc.scalar.activation(rrsb[64:128, :], rr[0:64, 256:512], AF.Copy)
            # y = av * rr * sigmoid(gate), written in-place over sg
            yt = ytp.tile([128, 256], F32, tag="yt")
            for hi in range(2):
                ysl_p = slice(64 * hi, 64 * hi + 64)
                nc.vector.tensor_mul(yt[ysl_p, :], avt[hi][0:64, :],
                                     rrsb[ysl_p, :])
                ysl = sg[ysl_p, c, b * 256:(b + 1) * 256]
                nc.vector.tensor_mul(ysl, ysl.bitcast(F32), yt[ysl_p, :])

        # software pipeline: scores/exp of iteration i+1 are emitted (and
        # run on PE/ACT) before the back half of iteration i, so the PE
        # never sits behind an exp.
        pend = None
        for c in range(NCH):
            w = proj_load_w(wg_d, c)
            for nh in range(2):
                proj_chunk(w, sg, c, nh, rope=False)
            for b in range(NB):
                cur = attn_front(b, c)
                if pend is not None:
                    attn_back(pend)
                pend = cur
        attn_back(pend)

        # ---- output projection (transposed): outT[of, r] = Wo.T @ y
        for oc in range(NCH):
            w = proj_load_w(wo_d, oc)
            for nh in range(2):
                ps = big_ps.tile([128, 512], F32, tag="big")
                for yc in range(NCH):
                    nc.tensor.matmul(
                        ps[:],
                        w[:, yc, :],
                        sg[:, yc, nh * 512:(nh + 1) * 512],
                        start=(yc == 0),
                        stop=(yc == NCH - 1),
                    )
                osb = opool.tile([128, 512], F32, tag="o")
                nc.scalar.activation(osb[:], ps[:], AF.Copy)
                nc.sync.dma_start(
                    outT_d[oc * 128:(oc + 1) * 128,
                           nh * 512:(nh + 1) * 512], osb[:])


def _build_nc():
    import concourse.bacc as bacc
    import concourse.mybir as mybir
    import concourse.tile as tile

    F32 = mybir.dt.float32
    nc = bacc.Bacc("TRN2", target_bir_lowering=False, debug=False)
    names_in = ["xT", "wq", "wk", "wv", "wg", "wo", "cos2", "sin2"]
    shapes_in = [[E, R], [E, E], [E, E], [E, E], [E, E], [E, E],
                 [128, R], [128, R]]
    ins = [
        nc.dram_tensor(n, s, F32, kind="ExternalInput").ap()
        for n, s in zip(names_in, shapes_in)
    ]
    outT = nc.dram_tensor("outT", [E, R], F32, kind="ExternalOutput").ap()
    with tile.TileContext(nc) as tc:
        emit(tc, [outT], ins)
    nc.compile()
    return nc


_NC_CACHE = {}


def host_prep(x, Wq, Wk, Wv, Wg, Wo, cos, sin):
    """Build the 8 per-core input maps."""
    x_flat = np.ascontiguousarray(x.reshape(B * S, E), dtype=np.float32)
    Wq = np.ascontiguousarray(Wq, dtype=np.float32)
    Wk = np.ascontiguousarray(Wk, dtype=np.float32)
    Wv = np.ascontiguousarray(Wv, dtype=np.float32)
    Wg = np.ascontiguousarray(Wg, dtype=np.float32)
    Wo = np.ascontiguousarray(Wo, dtype=np.float32)
    cos = np.asarray(cos, dtype=np.float32)
    sin = np.asarray(sin, dtype=np.float32)
    sign = np.where(np.arange(D) < D // 2, -1.0, 1.0).astype(np.float32)

    in_maps = []
    for cix in range(NCORES):
        rows = slice(cix * R, (cix + 1) * R)
        xT = np.ascontiguousarray(x_flat[rows].T)
        seq = (cix * R + np.arange(R)) % S
        cS = cos[seq]            # [R, D]
        sS = sin[seq] * sign     # [R, D] signed
        c2 = np.ascontiguousarray(np.tile(cS.T, (2, 1)))   # [128, R]
        s2 = np.ascontiguousarray(np.tile(sS.T, (2, 1)))   # [128, R]
        in_maps.append({
            "xT": xT, "wq": Wq, "wk": Wk, "wv": Wv, "wg": Wg, "wo": Wo,
            "cos2": c2, "sin2": s2,
        })
    return in_maps


def kernel_traced(x, Wq, Wk, Wv, Wg, Wo, cos, sin, block_size, trace=False,
                  **run_kwargs):
    assert int(block_size) == BLK
    from concourse import bass_utils

    if "nc" not in _NC_CACHE:
        _NC_CACHE["nc"] = _build_nc()
    nc = _NC_CACHE["nc"]

    in_maps = host_prep(x, Wq, Wk, Wv, Wg, Wo, cos, sin)
    res = bass_utils.run_bass_kernel_spmd(
        nc, in_maps, core_ids=list(range(NCORES)), trace=trace, **run_kwargs)
    out_flat = np.empty((B * S, E), dtype=np.float32)
    for cix in range(NCORES):
        out_flat[cix * R:(cix + 1) * R] = res.results[cix]["outT"].T
    return out_flat.reshape(B, S, E), res


def kernel(x, Wq, Wk, Wv, Wg, Wo, cos, sin, block_size):
    return kernel_traced(x, Wq, Wk, Wv, Wg, Wo, cos, sin, block_size)[0]


# revision 19
# speedup vs baseline: 1.1833x; 1.1793x over previous
"""Trainium2 Bass kernel for nn_BlockAttention (block-local attention with RoPE + gate).

Sharding: sequence-parallel over 8 cores. Flattened [B*S=8192, E] rows split into
8 contiguous shards of 1024 rows (4 blocks of 256; blocks never cross cores or
batch boundaries since 4096/256=16 blocks per batch, 4 per core).

Per-core layout strategy (features-on-partitions, "transposed" activations):
  - host pre-transposes the x shard to xT [E, R] so no on-chip transposes needed
  - qT/kT/gateT [E, R] = W.T @ x via matmul(lhsT=W_chunk, rhs=xT_chunk)  (fp32r)
  - v [R, E] natural via matmul(lhsT=xT_chunk, rhs=Wv_chunk)
  - RoPE applied on transposed q/k with host-prepared cos/sin tables
    (replicated per head-pair, rotate-sign folded into sin table)
  - block-local attention per (block, head) with transposed scores S_T[k,q]:
    exp on ScalarE (no max subtraction needed: |scores/8| < ~15), row-sums via
    M=1 ones-matmul on PE, AV via matmul(lhsT=v_block, rhs=expS_T),
    softmax normalize via K=1 ones-outer-product replicate matmul + DVE mul
  - gate: sigmoid on ScalarE, fused multiply on DVE
  - out projection back through Wo in transposed layout; host un-transposes
"""
import sys

sys.path.insert(0, "/opt/trn_rl_repo")
import numpy as np

B, S, E = 2, 4096, 1024
H, D = 16, 64
BLK = 256
NCORES = 8
R = (B * S) // NCORES   # 1024 rows per core
NB = R // BLK           # 4 blocks per core
NCH = E // 128          # 8 feature chunks of 128
SCALE = 1.0 / np.sqrt(D)


def emit(tc, outs, ins):
    """Emit the per-core program. ins/outs are DRAM APs:
    ins  = [xT, wq, wk, wv, wg, wo, cos2, sin2]
    outs = [outT]
    """
    from contextlib import ExitStack
    import concourse.mybir as mybir

    F32 = mybir.dt.float32
    F32R = mybir.dt.float32r
    F16 = mybir.dt.float16
    AF = mybir.ActivationFunctionType

    nc = tc.nc
    xT_d, wq_d, wk_d, wv_d, wg_d, wo_d, c2_d, s2_d = ins
    (outT_d,) = outs

    with ExitStack() as ctx:
        ep = ctx.enter_context
        consts = ep(tc.tile_pool(name="consts", bufs=1))
        big = ep(tc.tile_pool(name="big", bufs=1))
        wpool = ep(tc.tile_pool(name="wpool", bufs=3))
        wvpool = ep(tc.tile_pool(name="wvpool", bufs=1))
        ropet = ep(tc.tile_pool(name="ropet", bufs=3))
        espool = ep(tc.tile_pool(name="espool", bufs=4))
        smalls = ep(tc.tile_pool(name="smalls", bufs=2))
        rrsbp = ep(tc.tile_pool(name="rrsbp", bufs=2))
        ytp = ep(tc.tile_pool(name="ytp", bufs=2))
        opool = ep(tc.tile_pool(name="opool", bufs=2))
        # PSUM: 8 banks total, everything double-buffered.
        big_ps = ep(tc.tile_pool(name="big_ps", bufs=2, space="PSUM"))
        s_ps_p = ep(tc.tile_pool(name="s_ps_p", bufs=2, space="PSUM"))
        av_ps_p = ep(tc.tile_pool(name="av_ps_p", bufs=2, space="PSUM"))
        rr_ps_p = ep(tc.tile_pool(name="rr_ps_p", bufs=2, space="PSUM"))

        # ---- constants / inputs resident in SBUF
        xt = big.tile([128, NCH, R], F16)
        for kc in range(NCH):
            nc.sync.dma_start(xt[:, kc, :], xT_d[kc * 128:(kc + 1) * 128, :])
        c2 = consts.tile([128, R], F32)
        nc.sync.dma_start(c2[:], c2_d[:])
        s2 = consts.tile([128, R], F32)
        nc.sync.dma_start(s2[:], s2_d[:])
        onesf = consts.tile([128, 1], F32)
        nc.vector.memset(onesf[:], 1.0)
        ones = consts.tile([128, 1], F32R)
        nc.scalar.activation(ones[:], onesf[:], AF.Copy)
        onesrowf = consts.tile([1, 64], F32)
        nc.vector.memset(onesrowf[:], 1.0)
        onesrow = consts.tile([1, 64], F32R)
        nc.scalar.activation(onesrow[:], onesrowf[:], AF.Copy)

        qT = big.tile([128, NCH, R], F32R)
        kT = big.tile([128, NCH, R], F32R)
        # v holds 16 heads x (64 dims + a ones column) per row-chunk: the
        # ones column makes each AV matmul also emit the softmax row-sums
        # (output row 64) for free.
        v = big.tile([128, NCH, H * 65], F32R)
        ones16f = consts.tile([128, 16], F32)
        nc.vector.memset(ones16f[:], 1.0)
        # sg doubles as y: y1 multiplies the gate in-place (av*rr*sg),
        # and the out projection consumes it. fp16: it feeds the fp16
        # out-projection matmul.
        sg = big.tile([128, NCH, R], F16)

        # ---- one projection output chunk: 8-matmul psum group + drain
        def proj_chunk(w, dst, mc, nh, rope):
            ps = big_ps.tile([128, 512], F32, tag="big")
            for kc in range(NCH):
                nc.tensor.matmul(
                    ps[:],
                    w[:, kc, :],
                    xt[:, kc, nh * 512:(nh + 1) * 512],
                    start=(kc == 0),
                    stop=(kc == NCH - 1),
                )
            dstsl = dst[:, mc, nh * 512:(nh + 1) * 512]
            if rope:
                t = ropet.tile([128, 512], F32, tag="t")
                for h2 in (0, 64):
                    nc.scalar.activation(
                        t[h2:h2 + 32, :], ps[h2 + 32:h2 + 64, :], AF.Copy)
                    nc.scalar.activation(
                        t[h2 + 32:h2 + 64, :], ps[h2:h2 + 32, :], AF.Copy)
                nc.vector.tensor_mul(
                    dstsl, ps[:], c2[:, nh * 512:(nh + 1) * 512])
                nc.vector.tensor_mul(
                    t[:], t[:], s2[:, nh * 512:(nh + 1) * 512])
                nc.vector.tensor_add(dstsl, dstsl.bitcast(F32), t[:])
            else:
                nc.scalar.activation(dstsl, ps[:], AF.Sigmoid)

        def proj_load_w(w_d, mc):
            w = wpool.tile([128, NCH, 128], F16, tag="w")
            src = w_d.rearrange("(kc p) m -> p kc m", p=128)
            nc.sync.dma_start(w[:], src[:, :, mc * 128:(mc + 1) * 128])
            return w

        for mc in range(NCH):
            w = proj_load_w(wq_d, mc)
            for nh in range(2):
                proj_chunk(w, qT, mc, nh, rope=True)
        for mc in range(NCH):
            w = proj_load_w(wk_d, mc)
            for nh in range(2):
                proj_chunk(w, kT, mc, nh, rope=True)

        # ---- v projection (natural layout: rows on partitions)
        for nq in range(4):
            wvb = wvpool.tile([128, NCH, 256], F16, tag="wv")
            for kc in range(NCH):
                nc.sync.dma_start(
                    wvb[:, kc, :],
                    wv_d[kc * 128:(kc + 1) * 128, nq * 256:(nq + 1) * 256])
            for rc in range(NCH):
                ps = big_ps.tile([128, 512], F32, tag="big")
                for kc in range(NCH):
                    nc.tensor.matmul(
                        ps[:, 0:256],
                        xt[:, kc, rc * 128:(rc + 1) * 128],
                        wvb[:, kc, :],
                        start=(kc == 0),
                        stop=(kc == NCH - 1),
                    )
                vh = v[:, rc, :].rearrange("p (h t) -> p h t", t=65)
                nc.vector.tensor_copy(
                    vh[:, 4 * nq:4 * nq + 4, 0:64],
                    ps[:, 0:256].rearrange("p (h d) -> p h d", d=64))
        for rc in range(NCH):
            vh = v[:, rc, :].rearrange("p (h t) -> p h t", t=65)
            nc.scalar.activation(vh[:, :, 64], ones16f[:], AF.Copy)

        # ---- gate projection interleaved with block-local attention:
        # chunk c's gate lands just before attention needs sg[:, c, :],
        # and the gate matmuls keep PE dense while attention's ACT/DVE
        # chain (exp, recip, rr) runs.
        def attn_front(b, c):
            est = []
            for hi in range(2):
                pb = 64 * hi
                sps = s_ps_p.tile([128, 512], F32, tag="s")
                for kph in range(2):
                    nc.tensor.matmul(
                        sps[:, kph * 256:(kph + 1) * 256],
                        kT[pb:pb + 64, c,
                           b * 256 + kph * 128:b * 256 + (kph + 1) * 128],
                        qT[pb:pb + 64, c, b * 256:(b + 1) * 256],
                        start=True, stop=True,
                    )
                es = espool.tile([128, 512], F32R, tag="es")
                nc.scalar.activation(es[:], sps[:], AF.Exp,
                                     scale=float(SCALE))
                est.append(es)
            return (b, c, est)

        def attn_back(st):
            b, c, est = st
            recipf = smalls.tile([1, 512], F32, tag="recipf", bufs=1)
            sumst = smalls.tile([1, 512], F32, tag="sumst", bufs=1)
            avt = []
            for hi in range(2):
                es = est[hi]
                h = 2 * c + hi
                # AV + row-sums in one accumulation group (ones-padded V):
                # rows 0..63 = V.T @ expS, row 64 = column sums of expS.
                av = av_ps_p.tile([65, 256], F32, tag="av")
                for kph in range(2):
                    nc.tensor.matmul(
                        av[:],
                        v[:, 2 * b + kph, h * 65:(h + 1) * 65],
                        es[:, kph * 256:(kph + 1) * 256],
                        start=(kph == 0), stop=(kph == 1),
                    )
                avt.append(av)
                # reciprocal_approx_fast misreads PSUM at base partition 64
                # on HW, so stage the sums row through SBUF.
                nc.vector.tensor_copy(
                    sumst[0:1, hi * 256:(hi + 1) * 256], av[64:65, :])
            nc.vector.reciprocal_approx_fast(recipf[:], sumst[:])
            recip = smalls.tile([1, 512], F32R, tag="recip")
            nc.scalar.activation(recip[:], recipf[:], AF.Copy)
            rr = rr_ps_p.tile([64, 512], F32, tag="rr")
            for hi in range(2):
                nc.tensor.matmul(
                    rr[:, hi * 256:(hi + 1) * 256],
                    onesrow[:],
                    recip[0:1, hi * 256:(hi + 1) * 256],
                    start=True, stop=True,
                )
            rrsb = rrsbp.tile([128, 256], F32, tag="rrsb")
            nc.scalar.activation(rrsb[0:64, :], rr[0:64, 0:256], AF.Copy)
            nc.scalar.activation(rrsb[64:128, :], rr[0:64, 256:512], AF.Copy)
            # y = av * rr * sigmoid(gate), written in-place over sg
            yt = ytp.tile([128, 256], F16, tag="yt")
            for hi in range(2):
                ysl_p = slice(64 * hi, 64 * hi + 64)
                nc.vector.tensor_mul(yt[ysl_p, :], avt[hi][0:64, :],
                                     rrsb[ysl_p, :])
                ysl = sg[ysl_p, c, b * 256:(b + 1) * 256]
                nc.vector.tensor_mul(ysl, ysl, yt[ysl_p, :])

        # software pipeline: scores/exp of iteration i+1 are emitted (and
        # run on PE/ACT) before the back half of iteration i, so the PE
        # never sits behind an exp.
        pend = None
        for c in range(NCH):
            w = proj_load_w(wg_d, c)
            for nh in range(2):
                proj_chunk(w, sg, c, nh, rope=False)
            for b in range(NB):
                cur = attn_front(b, c)
                if pend is not None:
                    attn_back(pend)
                pend = cur
        attn_back(pend)

        # ---- output projection (transposed): outT[of, r] = Wo.T @ y
        for oc in range(NCH):
            w = proj_load_w(wo_d, oc)
            for nh in range(2):
                ps = big_ps.tile([128, 512], F32, tag="big")
                for yc in range(NCH):
                    nc.tensor.matmul(
                        ps[:],
                        w[:, yc, :],
                        sg[:, yc, nh * 512:(nh + 1) * 512],
                        start=(yc == 0),
                        stop=(yc == NCH - 1),
                    )
                osb = opool.tile([128, 512], F32, tag="o")
                nc.scalar.activation(osb[:], ps[:], AF.Copy)
                nc.sync.dma_start(
                    outT_d[oc * 128:(oc + 1) * 128,
                           nh * 512:(nh + 1) * 512], osb[:])


def _build_nc():
    import concourse.bacc as bacc
    import concourse.mybir as mybir
    import concourse.tile as tile

    F32 = mybir.dt.float32
    F16 = mybir.dt.float16
    nc = bacc.Bacc("TRN2", target_bir_lowering=False, debug=False)
    names_in = ["xT", "wq", "wk", "wv", "wg", "wo", "cos2", "sin2"]
    shapes_in = [[E, R], [E, E], [E, E], [E, E], [E, E], [E, E],
                 [128, R], [128, R]]
    dts_in = [F16, F16, F16, F16, F16, F16, F32, F32]
    ins = [
        nc.dram_tensor(n, s, dt, kind="ExternalInput").ap()
        for n, s, dt in zip(names_in, shapes_in, dts_in)
    ]
    outT = nc.dram_tensor("outT", [E, R], F32, kind="ExternalOutput").ap()
    with tile.TileContext(nc) as tc:
        emit(tc, [outT], ins)
    nc.compile()
    return nc


_NC_CACHE = {}


def host_prep(x, Wq, Wk, Wv, Wg, Wo, cos, sin):
    """Build the 8 per-core input maps."""
    x_flat = np.ascontiguousarray(x.reshape(B * S, E), dtype=np.float32)
    Wq = np.ascontiguousarray(Wq, dtype=np.float16)
    Wk = np.ascontiguousarray(Wk, dtype=np.float16)
    Wv = np.ascontiguousarray(Wv, dtype=np.float16)
    Wg = np.ascontiguousarray(Wg, dtype=np.float16)
    Wo = np.ascontiguousarray(Wo, dtype=np.float16)
    cos = np.asarray(cos, dtype=np.float32)
    sin = np.asarray(sin, dtype=np.float32)
    sign = np.where(np.arange(D) < D // 2, -1.0, 1.0).astype(np.float32)

    in_maps = []
    for cix in range(NCORES):
        rows = slice(cix * R, (cix + 1) * R)
        xT = np.ascontiguousarray(x_flat[rows].T.astype(np.float16))
        seq = (cix * R + np.arange(R)) % S
        cS = cos[seq]            # [R, D]
        sS = sin[seq] * sign     # [R, D] signed
        c2 = np.ascontiguousarray(np.tile(cS.T, (2, 1)))   # [128, R]
        s2 = np.ascontiguousarray(np.tile(sS.T, (2, 1)))   # [128, R]
        in_maps.append({
            "xT": xT, "wq": Wq, "wk": Wk, "wv": Wv, "wg": Wg, "wo": Wo,
            "cos2": c2, "sin2": s2,
        })
    return in_maps


def kernel_traced(x, Wq, Wk, Wv, Wg, Wo, cos, sin, block_size, trace=False,
                  **run_kwargs):
    assert int(block_size) == BLK
    from concourse import bass_utils

    if "nc" not in _NC_CACHE:
        _NC_CACHE["nc"] = _build_nc()
    nc = _NC_CACHE["nc"]

    in_maps = host_prep(x, Wq, Wk, Wv, Wg, Wo, cos, sin)
    res = bass_utils.run_bass_kernel_spmd(
        nc, in_maps, core_ids=list(range(NCORES)), trace=trace, **run_kwargs)
    out_flat = np.empty((B * S, E), dtype=np.float32)
    for cix in range(NCORES):
        out_flat[cix * R:(cix + 1) * R] = res.results[cix]["outT"].T
    return out_flat.reshape(B, S, E), res


def kernel(x, Wq, Wk, Wv, Wg, Wo, cos, sin, block_size):
    return kernel_traced(x, Wq, Wk, Wv, Wg, Wo, cos, sin, block_size)[0]


# revision 20
# speedup vs baseline: 1.2628x; 1.0671x over previous
"""Trainium2 Bass kernel for nn_BlockAttention (block-local attention with RoPE + gate).

Sharding: sequence-parallel over 8 cores. Flattened [B*S=8192, E] rows split into
8 contiguous shards of 1024 rows (4 blocks of 256; blocks never cross cores or
batch boundaries since 4096/256=16 blocks per batch, 4 per core).

Per-core layout strategy (features-on-partitions, "transposed" activations):
  - host pre-transposes the x shard to xT [E, R] so no on-chip transposes needed
  - qT/kT/gateT [E, R] = W.T @ x via matmul(lhsT=W_chunk, rhs=xT_chunk)  (fp32r)
  - v [R, E] natural via matmul(lhsT=xT_chunk, rhs=Wv_chunk)
  - RoPE applied on transposed q/k with host-prepared cos/sin tables
    (replicated per head-pair, rotate-sign folded into sin table)
  - block-local attention per (block, head) with transposed scores S_T[k,q]:
    exp on ScalarE (no max subtraction needed: |scores/8| < ~15), row-sums via
    M=1 ones-matmul on PE, AV via matmul(lhsT=v_block, rhs=expS_T),
    softmax normalize via K=1 ones-outer-product replicate matmul + DVE mul
  - gate: sigmoid on ScalarE, fused multiply on DVE
  - out projection back through Wo in transposed layout; host un-transposes
"""
import sys

sys.path.insert(0, "/opt/trn_rl_repo")
import numpy as np

B, S, E = 2, 4096, 1024
H, D = 16, 64
BLK = 256
NCORES = 8
R = (B * S) // NCORES   # 1024 rows per core
NB = R // BLK           # 4 blocks per core
NCH = E // 128          # 8 feature chunks of 128
SCALE = 1.0 / np.sqrt(D)


def emit(tc, outs, ins):
    """Emit the per-core program. ins/outs are DRAM APs:
    ins  = [xT, wq, wk, wv, wg, wo, cos2, sin2]
    outs = [outT]
    """
    from contextlib import ExitStack
    import concourse.mybir as mybir

    F32 = mybir.dt.float32
    F32R = mybir.dt.float32r
    F16 = mybir.dt.float16
    AF = mybir.ActivationFunctionType

    nc = tc.nc
    xT_d, wq_d, wk_d, wv_d, wg_d, wo_d, c2_d, s2_d = ins
    (outT_d,) = outs

    with ExitStack() as ctx:
        ep = ctx.enter_context
        consts = ep(tc.tile_pool(name="consts", bufs=1))
        big = ep(tc.tile_pool(name="big", bufs=1))
        wpool = ep(tc.tile_pool(name="wpool", bufs=3))
        wvpool = ep(tc.tile_pool(name="wvpool", bufs=1))
        ropet = ep(tc.tile_pool(name="ropet", bufs=3))
        espool = ep(tc.tile_pool(name="espool", bufs=4))
        smalls = ep(tc.tile_pool(name="smalls", bufs=2))
        rrsbp = ep(tc.tile_pool(name="rrsbp", bufs=2))
        ytp = ep(tc.tile_pool(name="ytp", bufs=2))
        opool = ep(tc.tile_pool(name="opool", bufs=2))
        # PSUM: 8 banks total, everything double-buffered.
        big_ps = ep(tc.tile_pool(name="big_ps", bufs=2, space="PSUM"))
        s_ps_p = ep(tc.tile_pool(name="s_ps_p", bufs=2, space="PSUM"))
        av_ps_p = ep(tc.tile_pool(name="av_ps_p", bufs=2, space="PSUM"))
        rr_ps_p = ep(tc.tile_pool(name="rr_ps_p", bufs=2, space="PSUM"))

        # ---- constants / inputs resident in SBUF
        xt = big.tile([128, NCH, R], F16)
        for kc in range(NCH):
            nc.sync.dma_start(xt[:, kc, :], xT_d[kc * 128:(kc + 1) * 128, :])
        c2 = consts.tile([128, R], F32)
        nc.sync.dma_start(c2[:], c2_d[:])
        s2 = consts.tile([128, R], F32)
        nc.sync.dma_start(s2[:], s2_d[:])
        onesf = consts.tile([128, 1], F32)
        nc.vector.memset(onesf[:], 1.0)
        ones = consts.tile([128, 1], F32R)
        nc.scalar.activation(ones[:], onesf[:], AF.Copy)
        onesrowf = consts.tile([1, 64], F32)
        nc.vector.memset(onesrowf[:], 1.0)
        onesrow = consts.tile([1, 64], F32R)
        nc.scalar.activation(onesrow[:], onesrowf[:], AF.Copy)

        qT = big.tile([128, NCH, R], F32R)
        kT = big.tile([128, NCH, R], F32R)
        # v holds 16 heads x (64 dims + a ones column) per row-chunk: the
        # ones column makes each AV matmul also emit the softmax row-sums
        # (output row 64) for free.
        v = big.tile([128, NCH, H * 65], F32R)
        ones16f = consts.tile([128, 16], F32)
        nc.vector.memset(ones16f[:], 1.0)
        # sg doubles as y: y1 multiplies the gate in-place (av*rr*sg),
        # and the out projection consumes it. fp16: it feeds the fp16
        # out-projection matmul.
        sg = big.tile([128, NCH, R], F16)

        # ---- one projection output chunk: 8-matmul psum group + drain
        def proj_chunk(w, dst, mc, nh, rope):
            ps = big_ps.tile([128, 512], F32, tag="big")
            for kc in range(NCH):
                nc.tensor.matmul(
                    ps[:],
                    w[:, kc, :],
                    xt[:, kc, nh * 512:(nh + 1) * 512],
                    start=(kc == 0),
                    stop=(kc == NCH - 1),
                )
            dstsl = dst[:, mc, nh * 512:(nh + 1) * 512]
            if rope:
                t = ropet.tile([128, 512], F32, tag="t")
                for h2 in (0, 64):
                    nc.scalar.activation(
                        t[h2:h2 + 32, :], ps[h2 + 32:h2 + 64, :], AF.Copy)
                    nc.scalar.activation(
                        t[h2 + 32:h2 + 64, :], ps[h2:h2 + 32, :], AF.Copy)
                nc.vector.tensor_mul(
                    dstsl, ps[:], c2[:, nh * 512:(nh + 1) * 512])
                nc.vector.tensor_mul(
                    t[:], t[:], s2[:, nh * 512:(nh + 1) * 512])
                nc.vector.tensor_add(dstsl, dstsl.bitcast(F32), t[:])
            else:
                nc.scalar.activation(dstsl, ps[:], AF.Sigmoid)

        def proj_load_w(w_d, mc):
            w = wpool.tile([128, NCH, 128], F16, tag="w")
            src = w_d.rearrange("(kc p) m -> p kc m", p=128)
            nc.sync.dma_start(w[:], src[:, :, mc * 128:(mc + 1) * 128])
            return w

        for mc in range(NCH):
            w = proj_load_w(wq_d, mc)
            for nh in range(2):
                proj_chunk(w, qT, mc, nh, rope=True)
        for mc in range(NCH):
            w = proj_load_w(wk_d, mc)
            for nh in range(2):
                proj_chunk(w, kT, mc, nh, rope=True)

        # ---- v projection (natural layout: rows on partitions)
        for nq in range(4):
            wvb = wvpool.tile([128, NCH, 256], F16, tag="wv")
            for kc in range(NCH):
                nc.sync.dma_start(
                    wvb[:, kc, :],
                    wv_d[kc * 128:(kc + 1) * 128, nq * 256:(nq + 1) * 256])
            for rc in range(NCH):
                ps = big_ps.tile([128, 512], F32, tag="big")
                for kc in range(NCH):
                    nc.tensor.matmul(
                        ps[:, 0:256],
                        xt[:, kc, rc * 128:(rc + 1) * 128],
                        wvb[:, kc, :],
                        start=(kc == 0),
                        stop=(kc == NCH - 1),
                    )
                vh = v[:, rc, :].rearrange("p (h t) -> p h t", t=65)
                nc.vector.tensor_copy(
                    vh[:, 4 * nq:4 * nq + 4, 0:64],
                    ps[:, 0:256].rearrange("p (h d) -> p h d", d=64))
        for rc in range(NCH):
            vh = v[:, rc, :].rearrange("p (h t) -> p h t", t=65)
            nc.scalar.activation(vh[:, :, 64], ones16f[:], AF.Copy)

        # ---- gate projection interleaved with block-local attention:
        # chunk c's gate lands just before attention needs sg[:, c, :],
        # and the gate matmuls keep PE dense while attention's ACT/DVE
        # chain (exp, recip, rr) runs.
        def attn_front(b, c):
            est = []
            for hi in range(2):
                pb = 64 * hi
                sps = s_ps_p.tile([128, 512], F32, tag="s")
                for kph in range(2):
                    nc.tensor.matmul(
                        sps[:, kph * 256:(kph + 1) * 256],
                        kT[pb:pb + 64, c,
                           b * 256 + kph * 128:b * 256 + (kph + 1) * 128],
                        qT[pb:pb + 64, c, b * 256:(b + 1) * 256],
                        start=True, stop=True,
                    )
                es = espool.tile([128, 512], F32R, tag="es")
                nc.scalar.activation(es[:], sps[:], AF.Exp,
                                     scale=float(SCALE))
                est.append(es)
            return (b, c, est)

        def attn_mid(st):
            b, c, est = st
            recipf = smalls.tile([1, 512], F32, tag="recipf")
            sumst = smalls.tile([1, 512], F32, tag="sumst")
            # both heads' AV share one psum bank: hi=0 in cols 0:256,
            # hi=1 in cols 256:512; row 64 = softmax row-sums (ones-padded V)
            av = av_ps_p.tile([65, 512], F32, tag="av")
            for hi in range(2):
                es = est[hi]
                h = 2 * c + hi
                for kph in range(2):
                    nc.tensor.matmul(
                        av[:, hi * 256:(hi + 1) * 256],
                        v[:, 2 * b + kph, h * 65:(h + 1) * 65],
                        es[:, kph * 256:(kph + 1) * 256],
                        start=(kph == 0), stop=(kph == 1),
                    )
            # reciprocal_approx_fast misreads PSUM at base partition 64
            # on HW, so stage the sums row through SBUF.
            nc.vector.tensor_copy(sumst[:], av[64:65, :])
            nc.vector.reciprocal_approx_fast(recipf[:], sumst[:])
            recip = smalls.tile([1, 512], F32R, tag="recip")
            nc.scalar.activation(recip[:], recipf[:], AF.Copy)
            return (b, c, av, recip)

        def attn_tail(st):
            b, c, av, recip = st
            rr = rr_ps_p.tile([64, 512], F32, tag="rr")
            for hi in range(2):
                nc.tensor.matmul(
                    rr[:, hi * 256:(hi + 1) * 256],
                    onesrow[:],
                    recip[0:1, hi * 256:(hi + 1) * 256],
                    start=True, stop=True,
                )
            rrsb = rrsbp.tile([128, 256], F32, tag="rrsb")
            nc.scalar.activation(rrsb[0:64, :], rr[0:64, 0:256], AF.Copy)
            nc.scalar.activation(rrsb[64:128, :], rr[0:64, 256:512], AF.Copy)
            # y = av * rr * sigmoid(gate), written in-place over sg
            yt = ytp.tile([128, 256], F16, tag="yt")
            for hi in range(2):
                ysl_p = slice(64 * hi, 64 * hi + 64)
                nc.vector.tensor_mul(yt[ysl_p, :],
                                     av[0:64, hi * 256:(hi + 1) * 256],
                                     rrsb[ysl_p, :])
                ysl = sg[ysl_p, c, b * 256:(b + 1) * 256]
                nc.vector.tensor_mul(ysl, ysl, yt[ysl_p, :])

        # software pipeline, 3 stages deep: scores/exp of iteration i,
        # AV+recip of i-1, rr/normalize/gate of i-2 — so the PE never waits
        # on the exp (stage A->B) or the reciprocal chain (stage B->C).
        p1 = p2 = None
        for c in range(NCH):
            w = proj_load_w(wg_d, c)
            for nh in range(2):
                proj_chunk(w, sg, c, nh, rope=False)
            for b in range(NB):
                cur = attn_front(b, c)
                if p1 is not None:
                    m = attn_mid(p1)
                    if p2 is not None:
                        attn_tail(p2)
                    p2 = m
                p1 = cur
        m = attn_mid(p1)
        attn_tail(p2)
        attn_tail(m)

        # ---- output projection (transposed): outT[of, r] = Wo.T @ y
        for oc in range(NCH):
            w = proj_load_w(wo_d, oc)
            for nh in range(2):
                ps = big_ps.tile([128, 512], F32, tag="big")
                for yc in range(NCH):
                    nc.tensor.matmul(
                        ps[:],
                        w[:, yc, :],
                        sg[:, yc, nh * 512:(nh + 1) * 512],
                        start=(yc == 0),
                        stop=(yc == NCH - 1),
                    )
                osb = opool.tile([128, 512], F32, tag="o")
                nc.scalar.activation(osb[:], ps[:], AF.Copy)
                nc.sync.dma_start(
                    outT_d[oc * 128:(oc + 1) * 128,
                           nh * 512:(nh + 1) * 512], osb[:])


def _build_nc():
    import concourse.bacc as bacc
    import concourse.mybir as mybir
    import concourse.tile as tile

    F32 = mybir.dt.float32
    F16 = mybir.dt.float16
    nc = bacc.Bacc("TRN2", target_bir_lowering=False, debug=False)
    names_in = ["xT", "wq", "wk", "wv", "wg", "wo", "cos2", "sin2"]
    shapes_in = [[E, R], [E, E], [E, E], [E, E], [E, E], [E, E],
                 [128, R], [128, R]]
    dts_in = [F16, F16, F16, F16, F16, F16, F32, F32]
    ins = [
        nc.dram_tensor(n, s, dt, kind="ExternalInput").ap()
        for n, s, dt in zip(names_in, shapes_in, dts_in)
    ]
    outT = nc.dram_tensor("outT", [E, R], F32, kind="ExternalOutput").ap()
    with tile.TileContext(nc) as tc:
        emit(tc, [outT], ins)
    nc.compile()
    return nc


_NC_CACHE = {}


def host_prep(x, Wq, Wk, Wv, Wg, Wo, cos, sin):
    """Build the 8 per-core input maps."""
    x_flat = np.ascontiguousarray(x.reshape(B * S, E), dtype=np.float32)
    Wq = np.ascontiguousarray(Wq, dtype=np.float16)
    Wk = np.ascontiguousarray(Wk, dtype=np.float16)
    Wv = np.ascontiguousarray(Wv, dtype=np.float16)
    Wg = np.ascontiguousarray(Wg, dtype=np.float16)
    Wo = np.ascontiguousarray(Wo, dtype=np.float16)
    cos = np.asarray(cos, dtype=np.float32)
    sin = np.asarray(sin, dtype=np.float32)
    sign = np.where(np.arange(D) < D // 2, -1.0, 1.0).astype(np.float32)

    in_maps = []
    for cix in range(NCORES):
        rows = slice(cix * R, (cix + 1) * R)
        xT = np.ascontiguousarray(x_flat[rows].T.astype(np.float16))
        seq = (cix * R + np.arange(R)) % S
        cS = cos[seq]            # [R, D]
        sS = sin[seq] * sign     # [R, D] signed
        c2 = np.ascontiguousarray(np.tile(cS.T, (2, 1)))   # [128, R]
        s2 = np.ascontiguousarray(np.tile(sS.T, (2, 1)))   # [128, R]
        in_maps.append({
            "xT": xT, "wq": Wq, "wk": Wk, "wv": Wv, "wg": Wg, "wo": Wo,
            "cos2": c2, "sin2": s2,
        })
    return in_maps


def kernel_traced(x, Wq, Wk, Wv, Wg, Wo, cos, sin, block_size, trace=False,
                  **run_kwargs):
    assert int(block_size) == BLK
    from concourse import bass_utils

    if "nc" not in _NC_CACHE:
        _NC_CACHE["nc"] = _build_nc()
    nc = _NC_CACHE["nc"]

    in_maps = host_prep(x, Wq, Wk, Wv, Wg, Wo, cos, sin)
    res = bass_utils.run_bass_kernel_spmd(
        nc, in_maps, core_ids=list(range(NCORES)), trace=trace, **run_kwargs)
    out_flat = np.empty((B * S, E), dtype=np.float32)
    for cix in range(NCORES):
        out_flat[cix * R:(cix + 1) * R] = res.results[cix]["outT"].T
    return out_flat.reshape(B, S, E), res


def kernel(x, Wq, Wk, Wv, Wg, Wo, cos, sin, block_size):
    return kernel_traced(x, Wq, Wk, Wv, Wg, Wo, cos, sin, block_size)[0]


# revision 21
# speedup vs baseline: 1.2660x; 1.0025x over previous
"""Trainium2 Bass kernel for nn_BlockAttention (block-local attention with RoPE + gate).

Sharding: sequence-parallel over 8 cores. Flattened [B*S=8192, E] rows split into
8 contiguous shards of 1024 rows (4 blocks of 256; blocks never cross cores or
batch boundaries since 4096/256=16 blocks per batch, 4 per core).

Per-core layout strategy (features-on-partitions, "transposed" activations):
  - host pre-transposes the x shard to xT [E, R] so no on-chip transposes needed
  - qT/kT/gateT [E, R] = W.T @ x via matmul(lhsT=W_chunk, rhs=xT_chunk)  (fp32r)
  - v [R, E] natural via matmul(lhsT=xT_chunk, rhs=Wv_chunk)
  - RoPE applied on transposed q/k with host-prepared cos/sin tables
    (replicated per head-pair, rotate-sign folded into sin table)
  - block-local attention per (block, head) with transposed scores S_T[k,q]:
    exp on ScalarE (no max subtraction needed: |scores/8| < ~15), row-sums via
    M=1 ones-matmul on PE, AV via matmul(lhsT=v_block, rhs=expS_T),
    softmax normalize via K=1 ones-outer-product replicate matmul + DVE mul
  - gate: sigmoid on ScalarE, fused multiply on DVE
  - out projection back through Wo in transposed layout; host un-transposes
"""
import sys

sys.path.insert(0, "/opt/trn_rl_repo")
import numpy as np

B, S, E = 2, 4096, 1024
H, D = 16, 64
BLK = 256
NCORES = 8
R = (B * S) // NCORES   # 1024 rows per core
NB = R // BLK           # 4 blocks per core
NCH = E // 128          # 8 feature chunks of 128
SCALE = 1.0 / np.sqrt(D)


def emit(tc, outs, ins):
    """Emit the per-core program. ins/outs are DRAM APs:
    ins  = [xT, wq, wk, wv, wg, wo, cos2, sin2]
    outs = [outT]
    """
    from contextlib import ExitStack
    import concourse.mybir as mybir

    F32 = mybir.dt.float32
    F32R = mybir.dt.float32r
    F16 = mybir.dt.float16
    AF = mybir.ActivationFunctionType

    nc = tc.nc
    xT_d, wq_d, wk_d, wv_d, wg_d, wo_d, c2_d, s2_d, wqr_d, wkr_d = ins
    (outT_d,) = outs

    with ExitStack() as ctx:
        ep = ctx.enter_context
        consts = ep(tc.tile_pool(name="consts", bufs=1))
        big = ep(tc.tile_pool(name="big", bufs=1))
        wpool = ep(tc.tile_pool(name="wpool", bufs=3))
        wvpool = ep(tc.tile_pool(name="wvpool", bufs=1))
        ropet = ep(tc.tile_pool(name="ropet", bufs=3))
        espool = ep(tc.tile_pool(name="espool", bufs=4))
        smalls = ep(tc.tile_pool(name="smalls", bufs=2))
        rrsbp = ep(tc.tile_pool(name="rrsbp", bufs=2))
        ytp = ep(tc.tile_pool(name="ytp", bufs=2))
        opool = ep(tc.tile_pool(name="opool", bufs=2))
        # PSUM: 8 banks total, everything double-buffered.
        big_ps = ep(tc.tile_pool(name="big_ps", bufs=2, space="PSUM"))
        s_ps_p = ep(tc.tile_pool(name="s_ps_p", bufs=2, space="PSUM"))
        av_ps_p = ep(tc.tile_pool(name="av_ps_p", bufs=2, space="PSUM"))
        rr_ps_p = ep(tc.tile_pool(name="rr_ps_p", bufs=2, space="PSUM"))

        # ---- constants / inputs resident in SBUF
        xt = big.tile([128, NCH, R], F16)
        for kc in range(NCH):
            nc.sync.dma_start(xt[:, kc, :], xT_d[kc * 128:(kc + 1) * 128, :])
        c2 = consts.tile([128, R], F32)
        nc.sync.dma_start(c2[:], c2_d[:])
        s2 = consts.tile([128, R], F32)
        nc.sync.dma_start(s2[:], s2_d[:])
        onesf = consts.tile([128, 1], F32)
        nc.vector.memset(onesf[:], 1.0)
        ones = consts.tile([128, 1], F32R)
        nc.scalar.activation(ones[:], onesf[:], AF.Copy)
        onesrowf = consts.tile([1, 64], F32)
        nc.vector.memset(onesrowf[:], 1.0)
        onesrow = consts.tile([1, 64], F32R)
        nc.scalar.activation(onesrow[:], onesrowf[:], AF.Copy)

        qT = big.tile([128, NCH, R], F32R)
        kT = big.tile([128, NCH, R], F32R)
        # v holds 16 heads x (64 dims + a ones column) per row-chunk: the
        # ones column makes each AV matmul also emit the softmax row-sums
        # (output row 64) for free.
        v = big.tile([128, NCH, H * 65], F32R)
        ones16f = consts.tile([128, 16], F32)
        nc.vector.memset(ones16f[:], 1.0)
        # sg doubles as y: y1 multiplies the gate in-place (av*rr*sg),
        # and the out projection consumes it. fp16: it feeds the fp16
        # out-projection matmul.
        sg = big.tile([128, NCH, R], F16)

        # ---- one projection output chunk: 8-matmul psum group + drain
        def mm_group(w, nh):
            ps = big_ps.tile([128, 512], F32, tag="big")
            for kc in range(NCH):
                nc.tensor.matmul(
                    ps[:],
                    w[:, kc, :],
                    xt[:, kc, nh * 512:(nh + 1) * 512],
                    start=(kc == 0),
                    stop=(kc == NCH - 1),
                )
            return ps

        def proj_chunk(w, dst, mc, nh, rope):
            ps = mm_group(w, nh)
            dstsl = dst[:, mc, nh * 512:(nh + 1) * 512]
            nc.scalar.activation(dstsl, ps[:], AF.Sigmoid)

        def proj_chunk_rope(w, wr, dst, mc, nh):
            # RoPE without any rotate-copies: the rotated projection comes
            # from a host-permuted weight copy, so the combine is 3 DVE ops.
            ps = mm_group(w, nh)
            psr = mm_group(wr, nh)
            dstsl = dst[:, mc, nh * 512:(nh + 1) * 512]
            t = ropet.tile([128, 512], F32, tag="t")
            nc.vector.tensor_mul(
                dstsl, ps[:], c2[:, nh * 512:(nh + 1) * 512])
            nc.vector.tensor_mul(
                t[:], psr[:], s2[:, nh * 512:(nh + 1) * 512])
            nc.vector.tensor_add(dstsl, dstsl.bitcast(F32), t[:])

        def proj_load_w(w_d, mc):
            w = wpool.tile([128, NCH, 128], F16, tag="w")
            src = w_d.rearrange("(kc p) m -> p kc m", p=128)
            nc.sync.dma_start(w[:], src[:, :, mc * 128:(mc + 1) * 128])
            return w

        for mc in range(NCH):
            w = proj_load_w(wq_d, mc)
            wr = proj_load_w(wqr_d, mc)
            for nh in range(2):
                proj_chunk_rope(w, wr, qT, mc, nh)
        for mc in range(NCH):
            w = proj_load_w(wk_d, mc)
            wr = proj_load_w(wkr_d, mc)
            for nh in range(2):
                proj_chunk_rope(w, wr, kT, mc, nh)

        # ---- v projection (natural layout: rows on partitions)
        for nq in range(4):
            wvb = wvpool.tile([128, NCH, 256], F16, tag="wv")
            for kc in range(NCH):
                nc.sync.dma_start(
                    wvb[:, kc, :],
                    wv_d[kc * 128:(kc + 1) * 128, nq * 256:(nq + 1) * 256])
            for rc in range(NCH):
                ps = big_ps.tile([128, 512], F32, tag="big")
                for kc in range(NCH):
                    nc.tensor.matmul(
                        ps[:, 0:256],
                        xt[:, kc, rc * 128:(rc + 1) * 128],
                        wvb[:, kc, :],
                        start=(kc == 0),
                        stop=(kc == NCH - 1),
                    )
                vh = v[:, rc, :].rearrange("p (h t) -> p h t", t=65)
                nc.vector.tensor_copy(
                    vh[:, 4 * nq:4 * nq + 4, 0:64],
                    ps[:, 0:256].rearrange("p (h d) -> p h d", d=64))
        for rc in range(NCH):
            vh = v[:, rc, :].rearrange("p (h t) -> p h t", t=65)
            nc.scalar.activation(vh[:, :, 64], ones16f[:], AF.Copy)

        # ---- gate projection interleaved with block-local attention:
        # chunk c's gate lands just before attention needs sg[:, c, :],
        # and the gate matmuls keep PE dense while attention's ACT/DVE
        # chain (exp, recip, rr) runs.
        def attn_front(b, c):
            est = []
            for hi in range(2):
                pb = 64 * hi
                sps = s_ps_p.tile([128, 512], F32, tag="s")
                for kph in range(2):
                    nc.tensor.matmul(
                        sps[:, kph * 256:(kph + 1) * 256],
                        kT[pb:pb + 64, c,
                           b * 256 + kph * 128:b * 256 + (kph + 1) * 128],
                        qT[pb:pb + 64, c, b * 256:(b + 1) * 256],
                        start=True, stop=True,
                    )
                es = espool.tile([128, 512], F32R, tag="es")
                nc.scalar.activation(es[:], sps[:], AF.Exp,
                                     scale=float(SCALE))
                est.append(es)
            return (b, c, est)

        def attn_mid(st):
            b, c, est = st
            recipf = smalls.tile([1, 512], F32, tag="recipf")
            sumst = smalls.tile([1, 512], F32, tag="sumst")
            # both heads' AV share one psum bank: hi=0 in cols 0:256,
            # hi=1 in cols 256:512; row 64 = softmax row-sums (ones-padded V)
            av = av_ps_p.tile([65, 512], F32, tag="av")
            for hi in range(2):
                es = est[hi]
                h = 2 * c + hi
                for kph in range(2):
                    nc.tensor.matmul(
                        av[:, hi * 256:(hi + 1) * 256],
                        v[:, 2 * b + kph, h * 65:(h + 1) * 65],
                        es[:, kph * 256:(kph + 1) * 256],
                        start=(kph == 0), stop=(kph == 1),
                    )
            # reciprocal_approx_fast misreads PSUM at base partition 64
            # on HW, so stage the sums row through SBUF.
            nc.vector.tensor_copy(sumst[:], av[64:65, :])
            nc.vector.reciprocal_approx_fast(recipf[:], sumst[:])
            recip = smalls.tile([1, 512], F32R, tag="recip")
            nc.scalar.activation(recip[:], recipf[:], AF.Copy)
            return (b, c, av, recip)

        def attn_tail(st):
            b, c, av, recip = st
            rr = rr_ps_p.tile([64, 512], F32, tag="rr")
            for hi in range(2):
                nc.tensor.matmul(
                    rr[:, hi * 256:(hi + 1) * 256],
                    onesrow[:],
                    recip[0:1, hi * 256:(hi + 1) * 256],
                    start=True, stop=True,
                )
            rrsb = rrsbp.tile([128, 256], F32, tag="rrsb")
            nc.scalar.activation(rrsb[0:64, :], rr[0:64, 0:256], AF.Copy)
            nc.scalar.activation(rrsb[64:128, :], rr[0:64, 256:512], AF.Copy)
            # y = av * rr * sigmoid(gate), written in-place over sg
            yt = ytp.tile([128, 256], F16, tag="yt")
            for hi in range(2):
                ysl_p = slice(64 * hi, 64 * hi + 64)
                nc.vector.tensor_mul(yt[ysl_p, :],
                                     av[0:64, hi * 256:(hi + 1) * 256],
                                     rrsb[ysl_p, :])
                ysl = sg[ysl_p, c, b * 256:(b + 1) * 256]
                nc.vector.tensor_mul(ysl, ysl, yt[ysl_p, :])

        # software pipeline, 3 stages deep: scores/exp of iteration i,
        # AV+recip of i-1, rr/normalize/gate of i-2 — so the PE never waits
        # on the exp (stage A->B) or the reciprocal chain (stage B->C).
        p1 = p2 = None
        for c in range(NCH):
            w = proj_load_w(wg_d, c)
            for nh in range(2):
                proj_chunk(w, sg, c, nh, rope=False)
            for b in range(NB):
                cur = attn_front(b, c)
                if p1 is not None:
                    m = attn_mid(p1)
                    if p2 is not None:
                        attn_tail(p2)
                    p2 = m
                p1 = cur
        m = attn_mid(p1)
        attn_tail(p2)
        attn_tail(m)

        # ---- output projection (transposed): outT[of, r] = Wo.T @ y
        for oc in range(NCH):
            w = proj_load_w(wo_d, oc)
            for nh in range(2):
                ps = big_ps.tile([128, 512], F32, tag="big")
                for yc in range(NCH):
                    nc.tensor.matmul(
                        ps[:],
                        w[:, yc, :],
                        sg[:, yc, nh * 512:(nh + 1) * 512],
                        start=(yc == 0),
                        stop=(yc == NCH - 1),
                    )
                osb = opool.tile([128, 512], F32, tag="o")
                nc.scalar.activation(osb[:], ps[:], AF.Copy)
                nc.sync.dma_start(
                    outT_d[oc * 128:(oc + 1) * 128,
                           nh * 512:(nh + 1) * 512], osb[:])


def _build_nc():
    import concourse.bacc as bacc
    import concourse.mybir as mybir
    import concourse.tile as tile

    F32 = mybir.dt.float32
    F16 = mybir.dt.float16
    nc = bacc.Bacc("TRN2", target_bir_lowering=False, debug=False)
    names_in = ["xT", "wq", "wk", "wv", "wg", "wo", "cos2", "sin2",
                "wqr", "wkr"]
    shapes_in = [[E, R], [E, E], [E, E], [E, E], [E, E], [E, E],
                 [128, R], [128, R], [E, E], [E, E]]
    dts_in = [F16, F16, F16, F16, F16, F16, F32, F32, F16, F16]
    ins = [
        nc.dram_tensor(n, s, dt, kind="ExternalInput").ap()
        for n, s, dt in zip(names_in, shapes_in, dts_in)
    ]
    outT = nc.dram_tensor("outT", [E, R], F32, kind="ExternalOutput").ap()
    with tile.TileContext(nc) as tc:
        emit(tc, [outT], ins)
    nc.compile()
    return nc


_NC_CACHE = {}


def host_prep(x, Wq, Wk, Wv, Wg, Wo, cos, sin):
    """Build the 8 per-core input maps."""
    x_flat = np.ascontiguousarray(x.reshape(B * S, E), dtype=np.float32)
    Wq = np.ascontiguousarray(Wq, dtype=np.float16)
    Wk = np.ascontiguousarray(Wk, dtype=np.float16)
    Wv = np.ascontiguousarray(Wv, dtype=np.float16)
    Wg = np.ascontiguousarray(Wg, dtype=np.float16)
    Wo = np.ascontiguousarray(Wo, dtype=np.float16)
    cos = np.asarray(cos, dtype=np.float32)
    sin = np.asarray(sin, dtype=np.float32)
    sign = np.where(np.arange(D) < D // 2, -1.0, 1.0).astype(np.float32)
    # column permutation for the pre-rotated projections:
    # perm[h*64 + d] = h*64 + (d + 32) % 64 (the sign lives in sin2)
    d_idx = np.arange(E)
    perm = (d_idx // D) * D + (d_idx % D + D // 2) % D
    Wqr = np.ascontiguousarray(Wq[:, perm])
    Wkr = np.ascontiguousarray(Wk[:, perm])

    in_maps = []
    for cix in range(NCORES):
        rows = slice(cix * R, (cix + 1) * R)
        xT = np.ascontiguousarray(x_flat[rows].T.astype(np.float16))
        seq = (cix * R + np.arange(R)) % S
        cS = cos[seq]            # [R, D]
        sS = sin[seq] * sign     # [R, D] signed
        c2 = np.ascontiguousarray(np.tile(cS.T, (2, 1)))   # [128, R]
        s2 = np.ascontiguousarray(np.tile(sS.T, (2, 1)))   # [128, R]
        in_maps.append({
            "xT": xT, "wq": Wq, "wk": Wk, "wv": Wv, "wg": Wg, "wo": Wo,
            "cos2": c2, "sin2": s2, "wqr": Wqr, "wkr": Wkr,
        })
    return in_maps


def kernel_traced(x, Wq, Wk, Wv, Wg, Wo, cos, sin, block_size, trace=False,
                  **run_kwargs):
    assert int(block_size) == BLK
    from concourse import bass_utils

    if "nc" not in _NC_CACHE:
        _NC_CACHE["nc"] = _build_nc()
    nc = _NC_CACHE["nc"]

    in_maps = host_prep(x, Wq, Wk, Wv, Wg, Wo, cos, sin)
    res = bass_utils.run_bass_kernel_spmd(
        nc, in_maps, core_ids=list(range(NCORES)), trace=trace, **run_kwargs)
    out_flat = np.empty((B * S, E), dtype=np.float32)
    for cix in range(NCORES):
        out_flat[cix * R:(cix + 1) * R] = res.results[cix]["outT"].T
    return out_flat.reshape(B, S, E), res


def kernel(x, Wq, Wk, Wv, Wg, Wo, cos, sin, block_size):
    return kernel_traced(x, Wq, Wk, Wv, Wg, Wo, cos, sin, block_size)[0]


# revision 24
# speedup vs baseline: 1.4705x; 1.1615x over previous
"""Trainium2 Bass kernel for nn_BlockAttention (block-local attention with RoPE + gate).

Sharding: sequence-parallel over 8 cores. Flattened [B*S=8192, E] rows split into
8 contiguous shards of 1024 rows (4 blocks of 256; blocks never cross cores or
batch boundaries since 4096/256=16 blocks per batch, 4 per core).

Per-core layout strategy (features-on-partitions, "transposed" activations):
  - host pre-transposes the x shard to xT [E, R] so no on-chip transposes needed
  - qT/kT/gateT [E, R] = W.T @ x via matmul(lhsT=W_chunk, rhs=xT_chunk)  (fp32r)
  - v [R, E] natural via matmul(lhsT=xT_chunk, rhs=Wv_chunk)
  - RoPE applied on transposed q/k with host-prepared cos/sin tables
    (replicated per head-pair, rotate-sign folded into sin table)
  - block-local attention per (block, head) with transposed scores S_T[k,q]:
    exp on ScalarE (no max subtraction needed: |scores/8| < ~15), row-sums via
    M=1 ones-matmul on PE, AV via matmul(lhsT=v_block, rhs=expS_T),
    softmax normalize via K=1 ones-outer-product replicate matmul + DVE mul
  - gate: sigmoid on ScalarE, fused multiply on DVE
  - out projection back through Wo in transposed layout; host un-transposes
"""
import sys

sys.path.insert(0, "/opt/trn_rl_repo")
import numpy as np

B, S, E = 2, 4096, 1024
H, D = 16, 64
BLK = 256
NCORES = 8
R = (B * S) // NCORES   # 1024 rows per core
NB = R // BLK           # 4 blocks per core
NCH = E // 128          # 8 feature chunks of 128
SCALE = 1.0 / np.sqrt(D)


def emit(tc, outs, ins):
    """Emit the per-core program. ins/outs are DRAM APs:
    ins  = [xT, wq, wk, wv, wg, wo, cos2, sin2]
    outs = [outT]
    """
    from contextlib import ExitStack
    import concourse.mybir as mybir

    F32 = mybir.dt.float32
    F32R = mybir.dt.float32r
    F16 = mybir.dt.float16
    AF = mybir.ActivationFunctionType

    nc = tc.nc
    xT_d, wq_d, wk_d, wv_d, wg_d, wo_d, c2_d, s2_d, wqr_d, wkr_d = ins
    (outT_d,) = outs

    with ExitStack() as ctx:
        ep = ctx.enter_context
        consts = ep(tc.tile_pool(name="consts", bufs=1))
        big = ep(tc.tile_pool(name="big", bufs=1))
        wpool = ep(tc.tile_pool(name="wpool", bufs=3))
        wvpool = ep(tc.tile_pool(name="wvpool", bufs=1))
        ropet = ep(tc.tile_pool(name="ropet", bufs=3))
        espool = ep(tc.tile_pool(name="espool", bufs=4))
        smalls = ep(tc.tile_pool(name="smalls", bufs=2))
        rrsbp = ep(tc.tile_pool(name="rrsbp", bufs=2))
        ytp = ep(tc.tile_pool(name="ytp", bufs=2))
        opool = ep(tc.tile_pool(name="opool", bufs=2))
        # PSUM: 8 banks total, everything double-buffered.
        big_ps = ep(tc.tile_pool(name="big_ps", bufs=2, space="PSUM"))
        s_ps_p = ep(tc.tile_pool(name="s_ps_p", bufs=2, space="PSUM"))
        av_ps_p = ep(tc.tile_pool(name="av_ps_p", bufs=2, space="PSUM"))
        rr_ps_p = ep(tc.tile_pool(name="rr_ps_p", bufs=2, space="PSUM"))

        # ---- constants / inputs resident in SBUF
        xt = big.tile([128, NCH, R], F16)
        for kc in range(NCH):
            nc.sync.dma_start(xt[:, kc, :], xT_d[kc * 128:(kc + 1) * 128, :])
        c2 = consts.tile([128, R], F32)
        nc.sync.dma_start(c2[:], c2_d[:])
        s2 = consts.tile([128, R], F32)
        nc.sync.dma_start(s2[:], s2_d[:])
        onesf = consts.tile([128, 1], F32)
        nc.vector.memset(onesf[:], 1.0)
        ones = consts.tile([128, 1], F32R)
        nc.scalar.activation(ones[:], onesf[:], AF.Copy)
        onesrowf = consts.tile([1, 64], F32)
        nc.vector.memset(onesrowf[:], 1.0)
        onesrow = consts.tile([1, 64], F32R)
        nc.scalar.activation(onesrow[:], onesrowf[:], AF.Copy)

        qT = big.tile([128, NCH, R], F32R)
        kT = big.tile([128, NCH, R], F32R)
        # v holds 16 heads x (64 dims + a ones column) per row-chunk: the
        # ones column makes each AV matmul also emit the softmax row-sums
        # (output row 64) for free.
        v = big.tile([128, NCH, H * 65], F32R)
        ones16f = consts.tile([128, 16], F32)
        nc.vector.memset(ones16f[:], 1.0)
        # sg doubles as y: y1 multiplies the gate in-place (av*rr*sg),
        # and the out projection consumes it. fp16: it feeds the fp16
        # out-projection matmul.
        sg = big.tile([128, NCH, R], F16)

        # ---- one projection output chunk: 8-matmul psum group + drain
        def mm_group(w, nh):
            ps = big_ps.tile([128, 512], F32, tag="big")
            for kc in range(NCH):
                nc.tensor.matmul(
                    ps[:],
                    w[:, kc, :],
                    xt[:, kc, nh * 512:(nh + 1) * 512],
                    start=(kc == 0),
                    stop=(kc == NCH - 1),
                )
            return ps

        def proj_chunk(w, dst, mc, nh, rope):
            ps = mm_group(w, nh)
            dstsl = dst[:, mc, nh * 512:(nh + 1) * 512]
            nc.scalar.activation(dstsl, ps[:], AF.Sigmoid)

        def proj_chunk_rope(w, wr, dst, mc, nh):
            # RoPE without any rotate-copies: the rotated projection comes
            # from a host-permuted weight copy, so the combine is 3 DVE ops.
            ps = mm_group(w, nh)
            psr = mm_group(wr, nh)
            dstsl = dst[:, mc, nh * 512:(nh + 1) * 512]
            t = ropet.tile([128, 512], F32, tag="t")
            nc.vector.tensor_mul(
                dstsl, ps[:], c2[:, nh * 512:(nh + 1) * 512])
            nc.vector.tensor_mul(
                t[:], psr[:], s2[:, nh * 512:(nh + 1) * 512])
            nc.vector.tensor_add(dstsl, dstsl.bitcast(F32), t[:])

        def proj_load_w(w_d, mc):
            w = wpool.tile([128, NCH, 128], F16, tag="w")
            src = w_d.rearrange("(kc p) m -> p kc m", p=128)
            nc.sync.dma_start(w[:], src[:, :, mc * 128:(mc + 1) * 128])
            return w


        def attn_front(b, c):
            est = []
            for hi in range(2):
                pb = 64 * hi
                sps = s_ps_p.tile([128, 512], F32, tag="s")
                for kph in range(2):
                    nc.tensor.matmul(
                        sps[:, kph * 256:(kph + 1) * 256],
                        kT[pb:pb + 64, c,
                           b * 256 + kph * 128:b * 256 + (kph + 1) * 128],
                        qT[pb:pb + 64, c, b * 256:(b + 1) * 256],
                        start=True, stop=True,
                    )
                es = espool.tile([128, 512], F32R, tag="es")
                nc.scalar.activation(es[:], sps[:], AF.Exp,
                                     scale=float(SCALE))
                est.append(es)
            return (b, c, est)

        def attn_mid(st):
            b, c, est = st
            recipf = smalls.tile([1, 512], F32, tag="recipf")
            sumst = smalls.tile([1, 512], F32, tag="sumst")
            # both heads' AV share one psum bank: hi=0 in cols 0:256,
            # hi=1 in cols 256:512; row 64 = softmax row-sums (ones-padded V)
            av = av_ps_p.tile([65, 512], F32, tag="av")
            for hi in range(2):
                es = est[hi]
                h = 2 * c + hi
                for kph in range(2):
                    nc.tensor.matmul(
                        av[:, hi * 256:(hi + 1) * 256],
                        v[:, 2 * b + kph, h * 65:(h + 1) * 65],
                        es[:, kph * 256:(kph + 1) * 256],
                        start=(kph == 0), stop=(kph == 1),
                    )
            # reciprocal_approx_fast misreads PSUM at base partition 64
            # on HW, so stage the sums row through SBUF.
            nc.vector.tensor_copy(sumst[:], av[64:65, :])
            nc.vector.reciprocal_approx_fast(recipf[:], sumst[:])
            recip = smalls.tile([1, 512], F32R, tag="recip")
            nc.scalar.activation(recip[:], recipf[:], AF.Copy)
            return (b, c, av, recip)

        def attn_tail(st):
            b, c, av, recip = st
            rr = rr_ps_p.tile([64, 512], F32, tag="rr")
            for hi in range(2):
                nc.tensor.matmul(
                    rr[:, hi * 256:(hi + 1) * 256],
                    onesrow[:],
                    recip[0:1, hi * 256:(hi + 1) * 256],
                    start=True, stop=True,
                )
            rrsb = rrsbp.tile([128, 256], F32, tag="rrsb")
            nc.scalar.activation(rrsb[0:64, :], rr[0:64, 0:256], AF.Copy)
            nc.scalar.activation(rrsb[64:128, :], rr[0:64, 256:512], AF.Copy)
            # y = av * rr * sigmoid(gate), written in-place over sg
            yt = ytp.tile([128, 256], F16, tag="yt")
            for hi in range(2):
                ysl_p = slice(64 * hi, 64 * hi + 64)
                nc.vector.tensor_mul(yt[ysl_p, :],
                                     av[0:64, hi * 256:(hi + 1) * 256],
                                     rrsb[ysl_p, :])
                ysl = sg[ysl_p, c, b * 256:(b + 1) * 256]
                nc.vector.tensor_mul(ysl, ysl, yt[ysl_p, :])

        for rc in range(NCH):
            vh = v[:, rc, :].rearrange("p (h t) -> p h t", t=65)
            nc.scalar.activation(vh[:, :, 64], ones16f[:], AF.Copy)

        # ---- fused main loop: per chunk c, emit the q/qrot/k/krot
        # projections for chunk c, the v quarter (every other c), the gate
        # chunk, then the (pipelined) attention iterations for chunk c.
        # The projection matmuls keep PE dense while attention's serial
        # ACT/DVE chains (exp -> sums -> recip -> rr) drain, which also
        # keeps the HAM clock-gate warm.
        p1 = p2 = None
        for c in range(NCH):
            w = proj_load_w(wq_d, c)
            wr = proj_load_w(wqr_d, c)
            for nh in range(2):
                proj_chunk_rope(w, wr, qT, c, nh)
            w = proj_load_w(wk_d, c)
            wr = proj_load_w(wkr_d, c)
            for nh in range(2):
                proj_chunk_rope(w, wr, kT, c, nh)
            if c % 2 == 0:
                nq = c // 2
                wvb = wvpool.tile([128, NCH, 256], F16, tag="wv")
                for kc in range(NCH):
                    nc.sync.dma_start(
                        wvb[:, kc, :],
                        wv_d[kc * 128:(kc + 1) * 128,
                             nq * 256:(nq + 1) * 256])
                for rc in range(NCH):
                    ps = big_ps.tile([128, 512], F32, tag="big")
                    for kc in range(NCH):
                        nc.tensor.matmul(
                            ps[:, 0:256],
                            xt[:, kc, rc * 128:(rc + 1) * 128],
                            wvb[:, kc, :],
                            start=(kc == 0),
                            stop=(kc == NCH - 1),
                        )
                    vh = v[:, rc, :].rearrange("p (h t) -> p h t", t=65)
                    nc.vector.tensor_copy(
                        vh[:, 4 * nq:4 * nq + 4, 0:64],
                        ps[:, 0:256].rearrange("p (h d) -> p h d", d=64))
            w = proj_load_w(wg_d, c)
            for nh in range(2):
                proj_chunk(w, sg, c, nh, rope=False)
            for b in range(NB):
                cur = attn_front(b, c)
                if p1 is not None:
                    m = attn_mid(p1)
                    if p2 is not None:
                        attn_tail(p2)
                    p2 = m
                p1 = cur
        m = attn_mid(p1)
        attn_tail(p2)
        attn_tail(m)

        # ---- output projection (transposed): outT[of, r] = Wo.T @ y
        for oc in range(NCH):
            w = proj_load_w(wo_d, oc)
            for nh in range(2):
                ps = big_ps.tile([128, 512], F32, tag="big")
                for yc in range(NCH):
                    nc.tensor.matmul(
                        ps[:],
                        w[:, yc, :],
                        sg[:, yc, nh * 512:(nh + 1) * 512],
                        start=(yc == 0),
                        stop=(yc == NCH - 1),
                    )
                osb = opool.tile([128, 512], F32, tag="o")
                nc.scalar.activation(osb[:], ps[:], AF.Copy)
                nc.sync.dma_start(
                    outT_d[oc * 128:(oc + 1) * 128,
                           nh * 512:(nh + 1) * 512], osb[:])


def _build_nc():
    import concourse.bacc as bacc
    import concourse.mybir as mybir
    import concourse.tile as tile

    F32 = mybir.dt.float32
    F16 = mybir.dt.float16
    nc = bacc.Bacc("TRN2", target_bir_lowering=False, debug=False)
    names_in = ["xT", "wq", "wk", "wv", "wg", "wo", "cos2", "sin2",
                "wqr", "wkr"]
    shapes_in = [[E, R], [E, E], [E, E], [E, E], [E, E], [E, E],
                 [128, R], [128, R], [E, E], [E, E]]
    dts_in = [F16, F16, F16, F16, F16, F16, F32, F32, F16, F16]
    ins = [
        nc.dram_tensor(n, s, dt, kind="ExternalInput").ap()
        for n, s, dt in zip(names_in, shapes_in, dts_in)
    ]
    outT = nc.dram_tensor("outT", [E, R], F32, kind="ExternalOutput").ap()
    with tile.TileContext(nc) as tc:
        emit(tc, [outT], ins)
    nc.compile()
    return nc


_NC_CACHE = {}


def host_prep(x, Wq, Wk, Wv, Wg, Wo, cos, sin):
    """Build the 8 per-core input maps."""
    x_flat = np.ascontiguousarray(x.reshape(B * S, E), dtype=np.float32)
    Wq = np.ascontiguousarray(Wq, dtype=np.float16)
    Wk = np.ascontiguousarray(Wk, dtype=np.float16)
    Wv = np.ascontiguousarray(Wv, dtype=np.float16)
    Wg = np.ascontiguousarray(Wg, dtype=np.float16)
    Wo = np.ascontiguousarray(Wo, dtype=np.float16)
    cos = np.asarray(cos, dtype=np.float32)
    sin = np.asarray(sin, dtype=np.float32)
    sign = np.where(np.arange(D) < D // 2, -1.0, 1.0).astype(np.float32)
    # column permutation for the pre-rotated projections:
    # perm[h*64 + d] = h*64 + (d + 32) % 64 (the sign lives in sin2)
    d_idx = np.arange(E)
    perm = (d_idx // D) * D + (d_idx % D + D // 2) % D
    Wqr = np.ascontiguousarray(Wq[:, perm])
    Wkr = np.ascontiguousarray(Wk[:, perm])

    in_maps = []
    for cix in range(NCORES):
        rows = slice(cix * R, (cix + 1) * R)
        xT = np.ascontiguousarray(x_flat[rows].T.astype(np.float16))
        seq = (cix * R + np.arange(R)) % S
        cS = cos[seq]            # [R, D]
        sS = sin[seq] * sign     # [R, D] signed
        c2 = np.ascontiguousarray(np.tile(cS.T, (2, 1)))   # [128, R]
        s2 = np.ascontiguousarray(np.tile(sS.T, (2, 1)))   # [128, R]
        in_maps.append({
            "xT": xT, "wq": Wq, "wk": Wk, "wv": Wv, "wg": Wg, "wo": Wo,
            "cos2": c2, "sin2": s2, "wqr": Wqr, "wkr": Wkr,
        })
    return in_maps


def kernel_traced(x, Wq, Wk, Wv, Wg, Wo, cos, sin, block_size, trace=False,
                  **run_kwargs):
    assert int(block_size) == BLK
    from concourse import bass_utils

    if "nc" not in _NC_CACHE:
        _NC_CACHE["nc"] = _build_nc()
    nc = _NC_CACHE["nc"]

    in_maps = host_prep(x, Wq, Wk, Wv, Wg, Wo, cos, sin)
    res = bass_utils.run_bass_kernel_spmd(
        nc, in_maps, core_ids=list(range(NCORES)), trace=trace, **run_kwargs)
    out_flat = np.empty((B * S, E), dtype=np.float32)
    for cix in range(NCORES):
        out_flat[cix * R:(cix + 1) * R] = res.results[cix]["outT"].T
    return out_flat.reshape(B, S, E), res


def kernel(x, Wq, Wk, Wv, Wg, Wo, cos, sin, block_size):
    return kernel_traced(x, Wq, Wk, Wv, Wg, Wo, cos, sin, block_size)[0]


# revision 25
# speedup vs baseline: 1.5793x; 1.0740x over previous
"""Trainium2 Bass kernel for nn_BlockAttention (block-local attention with RoPE + gate).

Sharding: sequence-parallel over 8 cores. Flattened [B*S=8192, E] rows split into
8 contiguous shards of 1024 rows (4 blocks of 256; blocks never cross cores or
batch boundaries since 4096/256=16 blocks per batch, 4 per core).

Per-core layout strategy (features-on-partitions, "transposed" activations):
  - host pre-transposes the x shard to xT [E, R] so no on-chip transposes needed
  - qT/kT/gateT [E, R] = W.T @ x via matmul(lhsT=W_chunk, rhs=xT_chunk)  (fp32r)
  - v [R, E] natural via matmul(lhsT=xT_chunk, rhs=Wv_chunk)
  - RoPE applied on transposed q/k with host-prepared cos/sin tables
    (replicated per head-pair, rotate-sign folded into sin table)
  - block-local attention per (block, head) with transposed scores S_T[k,q]:
    exp on ScalarE (no max subtraction needed: |scores/8| < ~15), row-sums via
    M=1 ones-matmul on PE, AV via matmul(lhsT=v_block, rhs=expS_T),
    softmax normalize via K=1 ones-outer-product replicate matmul + DVE mul
  - gate: sigmoid on ScalarE, fused multiply on DVE
  - out projection back through Wo in transposed layout; host un-transposes
"""
import sys

sys.path.insert(0, "/opt/trn_rl_repo")
import numpy as np

B, S, E = 2, 4096, 1024
H, D = 16, 64
BLK = 256
NCORES = 8
R = (B * S) // NCORES   # 1024 rows per core
NB = R // BLK           # 4 blocks per core
NCH = E // 128          # 8 feature chunks of 128
SCALE = 1.0 / np.sqrt(D)


def emit(tc, outs, ins):
    """Emit the per-core program. ins/outs are DRAM APs:
    ins  = [xT, wq, wk, wv, wg, wo, cos2, sin2]
    outs = [outT]
    """
    from contextlib import ExitStack
    import concourse.mybir as mybir

    F32 = mybir.dt.float32
    F32R = mybir.dt.float32r
    F16 = mybir.dt.float16
    AF = mybir.ActivationFunctionType

    nc = tc.nc
    xT_d, wq_d, wk_d, wv_d, wg_d, wo_d, c2_d, s2_d = ins
    (outT_d,) = outs

    with ExitStack() as ctx:
        ep = ctx.enter_context
        consts = ep(tc.tile_pool(name="consts", bufs=1))
        big = ep(tc.tile_pool(name="big", bufs=1))
        wpool = ep(tc.tile_pool(name="wpool", bufs=3))
        wvpool = ep(tc.tile_pool(name="wvpool", bufs=1))
        ropet = ep(tc.tile_pool(name="ropet", bufs=2))
        rawp = ep(tc.tile_pool(name="rawp", bufs=2))
        espool = ep(tc.tile_pool(name="espool", bufs=4))
        smalls = ep(tc.tile_pool(name="smalls", bufs=2))
        rrsbp = ep(tc.tile_pool(name="rrsbp", bufs=2))
        ytp = ep(tc.tile_pool(name="ytp", bufs=2))
        opool = ep(tc.tile_pool(name="opool", bufs=2))
        # PSUM: 8 banks total, everything double-buffered.
        big_ps = ep(tc.tile_pool(name="big_ps", bufs=2, space="PSUM"))
        s_ps_p = ep(tc.tile_pool(name="s_ps_p", bufs=2, space="PSUM"))
        av_ps_p = ep(tc.tile_pool(name="av_ps_p", bufs=2, space="PSUM"))
        rr_ps_p = ep(tc.tile_pool(name="rr_ps_p", bufs=2, space="PSUM"))

        # ---- constants / inputs resident in SBUF
        xt = big.tile([128, NCH, R], F16)
        for kc in range(NCH):
            nc.sync.dma_start(xt[:, kc, :], xT_d[kc * 128:(kc + 1) * 128, :])
        c2 = consts.tile([128, R], F32)
        nc.sync.dma_start(c2[:], c2_d[:])
        s2 = consts.tile([128, R], F32)
        nc.sync.dma_start(s2[:], s2_d[:])
        onesf = consts.tile([128, 1], F32)
        nc.vector.memset(onesf[:], 1.0)
        ones = consts.tile([128, 1], F32R)
        nc.scalar.activation(ones[:], onesf[:], AF.Copy)
        onesrowf = consts.tile([1, 64], F32)
        nc.vector.memset(onesrowf[:], 1.0)
        onesrow = consts.tile([1, 64], F32R)
        nc.scalar.activation(onesrow[:], onesrowf[:], AF.Copy)

        qT = big.tile([128, NCH, R], F32R)
        kT = big.tile([128, NCH, R], F32R)
        # v holds 16 heads x (64 dims + a ones column) per row-chunk: the
        # ones column makes each AV matmul also emit the softmax row-sums
        # (output row 64) for free.
        v = big.tile([128, NCH, H * 65], F32R)
        ones16f = consts.tile([128, 16], F32)
        nc.vector.memset(ones16f[:], 1.0)
        # sg doubles as y: y1 multiplies the gate in-place (av*rr*sg),
        # and the out projection consumes it. fp16: it feeds the fp16
        # out-projection matmul.
        sg = big.tile([128, NCH, R], F16)

        # ---- one projection output chunk: 8-matmul psum group + drain
        def mm_group(w, nh):
            ps = big_ps.tile([128, 512], F32, tag="big")
            for kc in range(NCH):
                nc.tensor.matmul(
                    ps[:],
                    w[:, kc, :],
                    xt[:, kc, nh * 512:(nh + 1) * 512],
                    start=(kc == 0),
                    stop=(kc == NCH - 1),
                )
            return ps

        def proj_chunk(w, dst, mc, nh, rope):
            ps = mm_group(w, nh)
            dstsl = dst[:, mc, nh * 512:(nh + 1) * 512]
            nc.scalar.activation(dstsl, ps[:], AF.Sigmoid)

        def proj_chunk_rope(w, dst, mc):
            # RoPE: the rotate (partition swap d <-> d+-32 within each head)
            # rides on the otherwise-idle DMA engines as 4 partition-shifted
            # SBUF->SBUF copies of the raw projection; sign lives in sin2.
            ps0 = mm_group(w, 0)
            ps1 = mm_group(w, 1)
            raw = rawp.tile([128, R], F32, tag="raw")
            nc.scalar.activation(raw[:, 0:512], ps0[:], AF.Copy)
            nc.scalar.activation(raw[:, 512:1024], ps1[:], AF.Copy)
            t = ropet.tile([128, R], F32, tag="t")
            for h2 in (0, 64):
                nc.sync.dma_start(t[h2:h2 + 32, :], raw[h2 + 32:h2 + 64, :])
                nc.gpsimd.dma_start(t[h2 + 32:h2 + 64, :], raw[h2:h2 + 32, :])
            dsl = dst[:, mc, :]
            nc.vector.tensor_mul(dsl[:, 0:512], ps0[:], c2[:, 0:512])
            nc.vector.tensor_mul(dsl[:, 512:1024], ps1[:], c2[:, 512:1024])
            nc.vector.tensor_mul(t[:], t[:], s2[:])
            nc.vector.tensor_add(dsl, dsl.bitcast(F32), t[:])

        def proj_load_w(w_d, mc):
            w = wpool.tile([128, NCH, 128], F16, tag="w")
            src = w_d.rearrange("(kc p) m -> p kc m", p=128)
            nc.sync.dma_start(w[:], src[:, :, mc * 128:(mc + 1) * 128])
            return w


        def attn_front(b, c):
            est = []
            for hi in range(2):
                pb = 64 * hi
                sps = s_ps_p.tile([128, 512], F32, tag="s")
                for kph in range(2):
                    nc.tensor.matmul(
                        sps[:, kph * 256:(kph + 1) * 256],
                        kT[pb:pb + 64, c,
                           b * 256 + kph * 128:b * 256 + (kph + 1) * 128],
                        qT[pb:pb + 64, c, b * 256:(b + 1) * 256],
                        start=True, stop=True,
                    )
                es = espool.tile([128, 512], F32R, tag="es")
                nc.scalar.activation(es[:], sps[:], AF.Exp,
                                     scale=float(SCALE))
                est.append(es)
            return (b, c, est)

        def attn_mid(st):
            b, c, est = st
            recipf = smalls.tile([1, 512], F32, tag="recipf")
            sumst = smalls.tile([1, 512], F32, tag="sumst")
            # both heads' AV share one psum bank: hi=0 in cols 0:256,
            # hi=1 in cols 256:512; row 64 = softmax row-sums (ones-padded V)
            av = av_ps_p.tile([65, 512], F32, tag="av")
            for hi in range(2):
                es = est[hi]
                h = 2 * c + hi
                for kph in range(2):
                    nc.tensor.matmul(
                        av[:, hi * 256:(hi + 1) * 256],
                        v[:, 2 * b + kph, h * 65:(h + 1) * 65],
                        es[:, kph * 256:(kph + 1) * 256],
                        start=(kph == 0), stop=(kph == 1),
                    )
            # reciprocal_approx_fast misreads PSUM at base partition 64
            # on HW, so stage the sums row through SBUF.
            nc.vector.tensor_copy(sumst[:], av[64:65, :])
            nc.vector.reciprocal_approx_fast(recipf[:], sumst[:])
            recip = smalls.tile([1, 512], F32R, tag="recip")
            nc.scalar.activation(recip[:], recipf[:], AF.Copy)
            return (b, c, av, recip)

        def attn_tail(st):
            b, c, av, recip = st
            rr = rr_ps_p.tile([64, 512], F32, tag="rr")
            for hi in range(2):
                nc.tensor.matmul(
                    rr[:, hi * 256:(hi + 1) * 256],
                    onesrow[:],
                    recip[0:1, hi * 256:(hi + 1) * 256],
                    start=True, stop=True,
                )
            rrsb = rrsbp.tile([128, 256], F32, tag="rrsb")
            nc.scalar.activation(rrsb[0:64, :], rr[0:64, 0:256], AF.Copy)
            nc.scalar.activation(rrsb[64:128, :], rr[0:64, 256:512], AF.Copy)
            # y = av * rr * sigmoid(gate), written in-place over sg
            yt = ytp.tile([128, 256], F16, tag="yt")
            for hi in range(2):
                ysl_p = slice(64 * hi, 64 * hi + 64)
                nc.vector.tensor_mul(yt[ysl_p, :],
                                     av[0:64, hi * 256:(hi + 1) * 256],
                                     rrsb[ysl_p, :])
                ysl = sg[ysl_p, c, b * 256:(b + 1) * 256]
                nc.vector.tensor_mul(ysl, ysl, yt[ysl_p, :])

        for rc in range(NCH):
            vh = v[:, rc, :].rearrange("p (h t) -> p h t", t=65)
            nc.scalar.activation(vh[:, :, 64], ones16f[:], AF.Copy)

        # ---- fused main loop: per chunk c, emit the q/qrot/k/krot
        # projections for chunk c, the v quarter (every other c), the gate
        # chunk, then the (pipelined) attention iterations for chunk c.
        # The projection matmuls keep PE dense while attention's serial
        # ACT/DVE chains (exp -> sums -> recip -> rr) drain, which also
        # keeps the HAM clock-gate warm.
        p1 = p2 = None
        for c in range(NCH):
            w = proj_load_w(wq_d, c)
            proj_chunk_rope(w, qT, c)
            w = proj_load_w(wk_d, c)
            proj_chunk_rope(w, kT, c)
            if c % 2 == 0:
                nq = c // 2
                wvb = wvpool.tile([128, NCH, 256], F16, tag="wv")
                for kc in range(NCH):
                    nc.sync.dma_start(
                        wvb[:, kc, :],
                        wv_d[kc * 128:(kc + 1) * 128,
                             nq * 256:(nq + 1) * 256])
                for rc in range(NCH):
                    ps = big_ps.tile([128, 512], F32, tag="big")
                    for kc in range(NCH):
                        nc.tensor.matmul(
                            ps[:, 0:256],
                            xt[:, kc, rc * 128:(rc + 1) * 128],
                            wvb[:, kc, :],
                            start=(kc == 0),
                            stop=(kc == NCH - 1),
                        )
                    vh = v[:, rc, :].rearrange("p (h t) -> p h t", t=65)
                    nc.vector.tensor_copy(
                        vh[:, 4 * nq:4 * nq + 4, 0:64],
                        ps[:, 0:256].rearrange("p (h d) -> p h d", d=64))
            w = proj_load_w(wg_d, c)
            for nh in range(2):
                proj_chunk(w, sg, c, nh, rope=False)
            for b in range(NB):
                cur = attn_front(b, c)
                if p1 is not None:
                    m = attn_mid(p1)
                    if p2 is not None:
                        attn_tail(p2)
                    p2 = m
                p1 = cur
        m = attn_mid(p1)
        attn_tail(p2)
        attn_tail(m)

        # ---- output projection (transposed): outT[of, r] = Wo.T @ y
        for oc in range(NCH):
            w = proj_load_w(wo_d, oc)
            for nh in range(2):
                ps = big_ps.tile([128, 512], F32, tag="big")
                for yc in range(NCH):
                    nc.tensor.matmul(
                        ps[:],
                        w[:, yc, :],
                        sg[:, yc, nh * 512:(nh + 1) * 512],
                        start=(yc == 0),
                        stop=(yc == NCH - 1),
                    )
                osb = opool.tile([128, 512], F32, tag="o")
                nc.scalar.activation(osb[:], ps[:], AF.Copy)
                nc.sync.dma_start(
                    outT_d[oc * 128:(oc + 1) * 128,
                           nh * 512:(nh + 1) * 512], osb[:])


def _build_nc():
    import concourse.bacc as bacc
    import concourse.mybir as mybir
    import concourse.tile as tile

    F32 = mybir.dt.float32
    F16 = mybir.dt.float16
    nc = bacc.Bacc("TRN2", target_bir_lowering=False, debug=False)
    names_in = ["xT", "wq", "wk", "wv", "wg", "wo", "cos2", "sin2"]
    shapes_in = [[E, R], [E, E], [E, E], [E, E], [E, E], [E, E],
                 [128, R], [128, R]]
    dts_in = [F16, F16, F16, F16, F16, F16, F32, F32]
    ins = [
        nc.dram_tensor(n, s, dt, kind="ExternalInput").ap()
        for n, s, dt in zip(names_in, shapes_in, dts_in)
    ]
    outT = nc.dram_tensor("outT", [E, R], F32, kind="ExternalOutput").ap()
    with tile.TileContext(nc) as tc:
        emit(tc, [outT], ins)
    nc.compile()
    return nc


_NC_CACHE = {}


def host_prep(x, Wq, Wk, Wv, Wg, Wo, cos, sin):
    """Build the 8 per-core input maps."""
    x_flat = np.ascontiguousarray(x.reshape(B * S, E), dtype=np.float32)
    Wq = np.ascontiguousarray(Wq, dtype=np.float16)
    Wk = np.ascontiguousarray(Wk, dtype=np.float16)
    Wv = np.ascontiguousarray(Wv, dtype=np.float16)
    Wg = np.ascontiguousarray(Wg, dtype=np.float16)
    Wo = np.ascontiguousarray(Wo, dtype=np.float16)
    cos = np.asarray(cos, dtype=np.float32)
    sin = np.asarray(sin, dtype=np.float32)
    sign = np.where(np.arange(D) < D // 2, -1.0, 1.0).astype(np.float32)

    in_maps = []
    for cix in range(NCORES):
        rows = slice(cix * R, (cix + 1) * R)
        xT = np.ascontiguousarray(x_flat[rows].T.astype(np.float16))
        seq = (cix * R + np.arange(R)) % S
        cS = cos[seq]            # [R, D]
        sS = sin[seq] * sign     # [R, D] signed
        c2 = np.ascontiguousarray(np.tile(cS.T, (2, 1)))   # [128, R]
        s2 = np.ascontiguousarray(np.tile(sS.T, (2, 1)))   # [128, R]
        in_maps.append({
            "xT": xT, "wq": Wq, "wk": Wk, "wv": Wv, "wg": Wg, "wo": Wo,
            "cos2": c2, "sin2": s2,
        })
    return in_maps


def kernel_traced(x, Wq, Wk, Wv, Wg, Wo, cos, sin, block_size, trace=False,
                  **run_kwargs):
    assert int(block_size) == BLK
    from concourse import bass_utils

    if "nc" not in _NC_CACHE:
        _NC_CACHE["nc"] = _build_nc()
    nc = _NC_CACHE["nc"]

    in_maps = host_prep(x, Wq, Wk, Wv, Wg, Wo, cos, sin)
    res = bass_utils.run_bass_kernel_spmd(
        nc, in_maps, core_ids=list(range(NCORES)), trace=trace, **run_kwargs)
    out_flat = np.empty((B * S, E), dtype=np.float32)
    for cix in range(NCORES):
        out_flat[cix * R:(cix + 1) * R] = res.results[cix]["outT"].T
    return out_flat.reshape(B, S, E), res


def kernel(x, Wq, Wk, Wv, Wg, Wo, cos, sin, block_size):
    return kernel_traced(x, Wq, Wk, Wv, Wg, Wo, cos, sin, block_size)[0]
